# revision 1
# baseline (speedup 1.0000x reference)
"""DGCNN forward on 8 Trainium2 NeuronCores (Bass/Tile).

Contract: kernel(**inputs) takes the FULL unsharded inputs of
reference.setup_inputs() and returns the FULL (4, 4096, 2) output.

Sharding: cores (2b, 2b+1) handle batch b; each core computes half the
rows (2048) of every edge-conv layer (NxN knn + top-20 gather + max) and
of the point-wise MLP. Between layers the pair exchanges its half of the
transposed feature map (plus a -|x|^2/2 row) via a pairwise AllGather.

Math: eval-BN is affine with positive scale s, and ReLU/max commute with
positive affine maps, so each edge-conv collapses to
    out[i] = ReLU( max_{j in knn20(i)} Y[j] + T[i] )
with Y = X @ (s*Wa).T  (gathered by knn index: 20 rows per point via
indirect DMA, max-combined across two 10-row gathers) and
T = X @ (s*(Wb-Wa)).T + (s*(b-m)+beta)  from a small matmul.
knn ranking uses pd = inner - xx_i/2 - xx_j/2 (= reference pd / 2, same
ordering), computed on the PE; top-20 selection runs on the Vector
engine (max8 / max_index / match_replace rounds).
"""

import sys

sys.path.insert(0, "/opt/trn_rl_repo")

import math
import os
from contextlib import ExitStack

import numpy as np

import concourse.bass as bass
import concourse.tile as tile
from concourse import bacc, mybir
from concourse.masks import make_identity

EPS = 1e-5
K = 20
N = 4096
P = 128
NTILES = N // P            # 32 point tiles per batch
NEG = -3.0e38
F32 = mybir.dt.float32
U32 = mybir.dt.uint32
AF = mybir.ActivationFunctionType
OP = mybir.AluOpType

# (C_in, O_out) per edge-conv layer
DIMS = [(3, 64), (64, 64), (64, 128), (128, 256)]

# False: every core computes its full batch (no collectives) - debug mode.
SHARD_HALVES = os.environ.get("DGCNN_SHARD", "1") == "1"
TOPK_MODE = os.environ.get("DGCNN_TOPK", "flat")
USE_F32R = os.environ.get("DGCNN_F32R", "0") == "1"
SCAN16 = os.environ.get("DGCNN_SCAN16", "0") == "1"
F16 = mybir.dt.float16
SCAN_DT = F16 if SCAN16 else mybir.dt.float32
SCAN_NEG = -60000.0 if SCAN16 else NEG

HALF = N // 2 if SHARD_HALVES else N
OWN_TILES = HALF // P      # 16 (32 in debug mode)


# --------------------------------------------------------------------------
# host-side weight folding
# --------------------------------------------------------------------------

def _fold(inputs):
    d = {}
    for l, (C, O) in enumerate(DIMS, 1):
        w = inputs[f"cw{l}"]            # (O, 2C)
        b = inputs[f"cb{l}"]
        g, be = inputs[f"g{l}"], inputs[f"b{l}"]
        m, v = inputs[f"m{l}"], inputs[f"v{l}"]
        s = g / np.sqrt(v + EPS)
        Wa, Wb = w[:, :C], w[:, C:]
        WaP = (s[:, None] * Wa).T.astype(np.float32)        # (C, O)
        W2P = (s[:, None] * (Wb - Wa)).T.astype(np.float32)  # (C, O)
        cP = (s * (b - m) + be).astype(np.float32)           # (O,)
        d[f"wa{l}"] = np.ascontiguousarray(WaP)
        if l < 4:
            d[f"w2{l}"] = np.ascontiguousarray(
                np.concatenate([W2P, cP[None, :]], 0))       # (C+1, O)
        else:
            d["w24a"] = np.ascontiguousarray(W2P[0:64])      # (64, 256)
            d["w24b"] = np.ascontiguousarray(
                np.concatenate([W2P[64:128], cP[None, :]], 0))  # (65, 256)

    def fold_lin(w, b, g, be, m, v):
        s = g / np.sqrt(v + EPS)
        return ((s[:, None] * w).T.astype(np.float32),
                (s * (b - m) + be).astype(np.float32))

    L1, c1 = fold_lin(inputs["l1w"], inputs["l1b"], inputs["f1g"],
                      inputs["f1b"], inputs["f1m"], inputs["f1v"])
    L2, c2 = fold_lin(inputs["l2w"], inputs["l2b"], inputs["f2g"],
                      inputs["f2b"], inputs["f2m"], inputs["f2v"])
    L3 = inputs["l3w"].T.astype(np.float32)                  # (256, 2)
    c3 = inputs["l3b"].astype(np.float32)

    d["mw1"] = np.ascontiguousarray(
        L1.reshape(4, 128, 1024).transpose(1, 0, 2).reshape(128, 4096))
    d["mc1"] = np.ascontiguousarray(c1.reshape(8, 128).T)    # [128, 8]
    d["mw2"] = np.ascontiguousarray(
        L2.reshape(8, 128, 256).transpose(1, 0, 2).reshape(128, 2048))
    d["mc2"] = np.ascontiguousarray(c2.reshape(2, 128).T)    # [128, 2]
    d["mw3"] = np.ascontiguousarray(
        L3.reshape(2, 128, 2).transpose(1, 0, 2).reshape(128, 4))
    d["mc3"] = np.ascontiguousarray(c3[:, None])             # [2, 1]
    return d


def make_core_inputs(inputs):
    """Per-core input dicts (len 8). Core c -> batch c//2, half c%2."""
    shared = _fold(inputs)
    x = np.asarray(inputs["x"], np.float32)          # (B, 4096, 3)
    B = x.shape[0]
    per_core = []
    for c in range(8):
        b = (c // 2) % B if SHARD_HALVES else c % B
        h = c % 2 if SHARD_HALVES else 0
        xb = x[b]
        xx = (xb * xb).sum(-1)
        xaug = np.concatenate([xb.T, -0.5 * xx[None, :]], 0).astype(np.float32)
        d = dict(shared)
        d["xaug1"] = np.ascontiguousarray(xaug)                      # (4, 4096)
        d["xown1"] = np.ascontiguousarray(xaug[:, h * HALF:(h + 1) * HALF])
        d["negxx1"] = np.ascontiguousarray(
            d["xown1"][3].reshape(HALF // 128, 128).T)
        per_core.append(d)
    return per_core


# --------------------------------------------------------------------------
# device program
# --------------------------------------------------------------------------

def build_program():
    nc = bacc.Bacc(None)
    ins = {}

    def einp(name, shape):
        ins[name] = nc.dram_tensor(name, shape, F32, kind="ExternalInput")
        return ins[name]

    einp("xaug1", [4, N])
    einp("xown1", [4, HALF])
    einp("negxx1", [128, HALF // 128])
    for l, (C, O) in enumerate(DIMS, 1):
        einp(f"wa{l}", [C, O])
        if l < 4:
            einp(f"w2{l}", [C + 1, O])
    einp("w24a", [64, 256])
    einp("w24b", [65, 256])
    einp("mw1", [128, 4096])
    einp("mc1", [128, 8])
    einp("mw2", [128, 2048])
    einp("mc2", [128, 2])
    einp("mw3", [128, 4])
    einp("mc3", [2, 1])

    out_d = nc.dram_tensor("out", [HALF, 2], F32, kind="ExternalOutput")

    with tile.TileContext(nc) as tc:
        with ExitStack() as ctx:
            _build_tile_graph(ctx, tc, ins, out_d)

    nc.compile()
    return nc


def _build_tile_graph(ctx, tc, ins, out_d):
    nc = tc.nc

    nb = 2 if SHARD_HALVES else 1
    const = ctx.enter_context(tc.tile_pool(name="const", bufs=1))
    big = ctx.enter_context(tc.tile_pool(name="big", bufs=1))
    pdp = ctx.enter_context(tc.tile_pool(name="pdp", bufs=3 if SHARD_HALVES else 1))
    stg = ctx.enter_context(tc.tile_pool(name="stg", bufs=nb))
    gat = ctx.enter_context(tc.tile_pool(name="gat", bufs=1))
    psum = ctx.enter_context(tc.tile_pool(name="psum", bufs=2, space="PSUM"))
    dram = ctx.enter_context(tc.tile_pool(name="dram", bufs=1, space="DRAM"))

    def ps_pd():
        return psum.tile([P, 512], F32, tag="pspd", name="pspd")

    def ps_yt():
        return psum.tile([P, 256], F32, tag="psY", name="psY")

    def ps_misc():
        return psum.tile([P, 512], F32, tag="psmisc", name="psmisc")

    ident = const.tile([P, P], F32)
    make_identity(nc, ident[:])

    wa_sb, w2_sb = {}, {}
    for l, (C, O) in enumerate(DIMS, 1):
        wa_sb[l] = const.tile([C, O], F32, tag=f"wa{l}", name=f"wa{l}sb")
        nc.sync.dma_start(wa_sb[l][:], ins[f"wa{l}"][:])
        if l < 4:
            w2_sb[l] = const.tile([C + 1, O], F32, tag=f"w2{l}", name=f"w2{l}sb")
            nc.sync.dma_start(w2_sb[l][:], ins[f"w2{l}"][:])
    w24a = const.tile([64, 256], F32)
    nc.sync.dma_start(w24a[:], ins["w24a"][:])
    w24b = const.tile([65, 256], F32)
    nc.sync.dma_start(w24b[:], ins["w24b"][:])

    # feature-major inputs: xaug rows 0..C-1 = x^T, row C = -|x|^2/2
    xaug_tag = {1: "xaug_a", 2: "xaug_b", 3: "xaug_a", 4: "xaug_b"}
    xaug = big.tile([P, N], F32, tag=xaug_tag[1])
    nc.sync.dma_start(xaug[0:4, :], ins["xaug1"][:])
    xown1 = big.tile([P, HALF], F32, tag="ownD")
    nc.sync.dma_start(xown1[0:4, :], ins["xown1"][:])

    # retained own-half feature-major activations (MLP k-chunks)
    tileA = big.tile([P, HALF], F32, tag="mlpA")  # x1T (rows 0:64) | x2T (64:128)
    tileB = big.tile([P, HALF], F32, tag="mlpB")  # x3T
    tileC = big.tile([P, HALF], F32, tag="mlpC")  # x4T rows 0:128
    tileD = big.tile([P, HALF], F32, tag="ownD")  # x4T rows 128:256 (reuses xown1)
    # (tile, row_offset) holding each layer's own-half input, feature-major
    xown_map = {1: (xown1, 0), 2: (tileA, 0), 3: (tileA, 64), 4: (tileB, 0)}

    negxx_t = None        # [P, OWN_TILES] per-tile -xx/2 bias (current layer)
    xaug4b = None

    for l, (C, O) in enumerate(DIMS, 1):
        # ---- stage A: Y' for the whole batch -> DRAM ----
        ydram = dram.tile([N, O], F32, tag=f"yd{l}")
        for j in range(NTILES):
            ps = ps_yt()
            nc.tensor.matmul(ps[:, 0:O], lhsT=xaug[0:C, j * P:(j + 1) * P],
                             rhs=wa_sb[l][:], start=True, stop=True)
            ysb = stg.tile([P, 256], F32, tag="ysb")
            nc.scalar.copy(ysb[:, 0:O], ps[:, 0:O])
            nc.sync.dma_start(ydram[j * P:(j + 1) * P, :], ysb[:, 0:O])

        if l == 1:
            negxx_t = stg.tile([P, NTILES], F32, tag="negxx")
            nc.sync.dma_start(negxx_t[:, 0:OWN_TILES], ins["negxx1"][:])

        if l < 4:
            eg_in = dram.tile([O + 1, HALF], F32, tag=f"egin{l}")
            if SHARD_HALVES:
                eg_out = dram.tile([2 * (O + 1), HALF], F32, tag=f"egout{l}")
            xxrow_sb = big.tile([1, HALF], F32, tag="xxrow")

        own_src, own_ro = xown_map[l]

        # ---- stage B: per own row-tile ----
        for t in range(OWN_TILES):
            # lhsT staging: rows 0..C-1 = x^T own slice, row C = ones
            lt = stg.tile([P, P], F32, tag="lhsT")
            if C < 32:
                nc.gpsimd.memset(lt[0:32, :], 1.0)
            elif l < 4:
                nc.gpsimd.memset(lt[C:C + 1, :], 1.0)
            nc.scalar.copy(lt[0:C if l < 4 else 64, :],
                           own_src[own_ro:own_ro + (C if l < 4 else 64),
                                   t * P:(t + 1) * P])
            kk = C + 1 if l < 4 else 64
            if l == 4:
                # K-split: second chunk carries rows 64:128 plus a ones row
                # (for cP / the -xx_j/2 row on the rhs side)
                lt4b = stg.tile([P, P], F32, tag="lhsT4b")
                nc.gpsimd.memset(lt4b[64:65, :], 1.0)
                nc.scalar.copy(lt4b[0:64, :],
                               own_src[64:128, t * P:(t + 1) * P])

            # T' = x @ W2P + cP  (cP via the ones row)
            psT = ps_yt()
            if l < 4:
                nc.tensor.matmul(psT[:, 0:O], lhsT=lt[0:kk, :], rhs=w2_sb[l][:],
                                 start=True, stop=True)
            else:
                nc.tensor.matmul(psT[:, 0:O], lhsT=lt[0:64, :], rhs=w24a[:],
                                 start=True, stop=False)
                nc.tensor.matmul(psT[:, 0:O], lhsT=lt4b[0:65, :], rhs=w24b[:],
                                 start=False, stop=True)
            tsb = stg.tile([P, 256], F32, tag="tsb")
            nc.scalar.copy(tsb[:, 0:O], psT[:, 0:O])

            # pd row-block
            pd = pdp.tile([P, N], SCAN_DT, tag="pd")
            for cc in range(8):
                ppd = ps_pd()
                sl = slice(cc * 512, (cc + 1) * 512)
                if l < 4:
                    nc.tensor.matmul(ppd[:], lhsT=lt[0:kk, :],
                                     rhs=xaug[0:kk, sl],
                                     start=True, stop=True)
                else:
                    nc.tensor.matmul(ppd[:], lhsT=lt[0:64, :],
                                     rhs=xaug[0:64, sl],
                                     start=True, stop=False)
                    nc.tensor.matmul(ppd[:], lhsT=lt4b[0:65, :],
                                     rhs=xaug4b[0:65, sl],
                                     start=False, stop=True)
                nc.scalar.activation(pd[:, sl], ppd[:], AF.Identity,
                                     bias=negxx_t[:, t:t + 1], scale=1.0)

            # top-20 neighbour indices
            idxs = stg.tile([P, 24], U32, tag="idxs")
            vals = stg.tile([P, 24], SCAN_DT, tag="vals")
            for r in range(3):
                v8 = vals[:, r * 8:r * 8 + 8]
                i8 = idxs[:, r * 8:r * 8 + 8]
                nc.vector.max(out=v8, in_=pd[:])
                nc.vector.max_index(out=i8, in_max=v8, in_values=pd[:])
                if r < 2:
                    nc.vector.match_replace(out=pd[:], in_to_replace=v8,
                                            in_values=pd[:], imm_value=SCAN_NEG)

            # gather the 20 neighbour Y' rows (HW indirect DMA honours ONE
            # dynamic offset per partition per instruction), then max over k
            G = gat.tile([P, 20 * O], F32, tag="G")
            for k in range(20):
                nc.gpsimd.indirect_dma_start(
                    out=G[:, k * O:(k + 1) * O], out_offset=None, in_=ydram[:],
                    in_offset=bass.IndirectOffsetOnAxis(ap=idxs[:, k:k + 1],
                                                        axis=0))
            Mx = stg.tile([P, 256], F32, tag="Mx")
            nc.vector.tensor_reduce(
                out=Mx[:, 0:O],
                in_=G[:].rearrange("p (s o) -> p o s", s=20, o=O),
                axis=mybir.AxisListType.X, op=OP.max)

            # out = ReLU(Mx + T')
            pre = stg.tile([P, 256], F32, tag="pre")
            nc.vector.tensor_add(pre[:, 0:O], Mx[:, 0:O], tsb[:, 0:O])
            opm = stg.tile([P, 256], F32, tag="opm")
            nc.scalar.activation(opm[:, 0:O], pre[:, 0:O], AF.Relu)

            # transpose to feature-major, retain own half, stage exchange
            if l < 4:
                ptr = ps_misc()
                nc.tensor.transpose(ptr[0:O, 0:P], opm[:, 0:O], ident[:])
                if l == 1:
                    dst = tileA[0:64, t * P:(t + 1) * P]
                elif l == 2:
                    dst = tileA[64:128, t * P:(t + 1) * P]
                else:
                    dst = tileB[0:128, t * P:(t + 1) * P]
                nc.scalar.copy(dst, ptr[0:O, 0:P])
                nc.sync.dma_start(eg_in[0:O, t * P:(t + 1) * P], dst)

                # -|x_new|^2/2 per point
                sq = stg.tile([P, 256], F32, tag="sq")
                xxc = stg.tile([P, 2], F32, tag="xxc")
                nc.scalar.activation(sq[:, 0:O], opm[:, 0:O], AF.Square,
                                     scale=float(math.sqrt(0.5)),
                                     accum_out=xxc[:, 0:1])
                nc.scalar.activation(xxc[:, 1:2], xxc[:, 0:1], AF.Identity,
                                     scale=-1.0)
                pxr = ps_misc()
                nc.tensor.transpose(pxr[0:1, 0:P], xxc[:, 1:2], ident[:])
                nc.scalar.copy(xxrow_sb[:, t * P:(t + 1) * P], pxr[0:1, 0:P])
            else:
                ptr = ps_misc()
                nc.tensor.transpose(ptr[0:P, 0:P], opm[:, 0:128], ident[:])
                nc.scalar.copy(tileC[:, t * P:(t + 1) * P], ptr[0:P, 0:P])
                ptr2 = ps_misc()
                nc.tensor.transpose(ptr2[0:P, 0:P], opm[:, 128:256], ident[:])
                nc.scalar.copy(tileD[:, t * P:(t + 1) * P], ptr2[0:P, 0:P])

        # ---- stage C: exchange halves, assemble next layer's inputs ----
        if l < 4:
            nc.sync.dma_start(eg_in[O:O + 1, :], xxrow_sb[:])
            if SHARD_HALVES:
                nc.gpsimd.collective_compute(
                    "AllGather", OP.bypass,
                    replica_groups=[[0, 1], [2, 3], [4, 5], [6, 7]],
                    ins=[eg_in.opt()], outs=[eg_out.opt()])
                src, blocks = eg_out, [(0, 0), (O + 1, HALF)]
            else:
                src, blocks = eg_in, [(0, 0)]

            xaug_next = big.tile([P, N], F32, tag=xaug_tag[l + 1])
            if l < 3:
                for (srow, cbase) in blocks:
                    nc.sync.dma_start(xaug_next[0:O, cbase:cbase + HALF],
                                      src[srow:srow + O, :])
                    nc.sync.dma_start(xaug_next[O:O + 1, cbase:cbase + HALF],
                                      src[srow + O:srow + O + 1, :])
            else:
                # layer-4 rhs K-chunks: xaug4 rows 0:64 (via xaug_next) and
                # xaug4b = rows 64:128 plus the -xx/2 row
                xaug4b = big.tile([P, N], F32, tag="xaug_a")
                for (srow, cbase) in blocks:
                    nc.sync.dma_start(xaug_next[0:O, cbase:cbase + HALF],
                                      src[srow:srow + O, :])
                    nc.sync.dma_start(xaug4b[0:64, cbase:cbase + HALF],
                                      src[srow + 64:srow + 128, :])
                    nc.sync.dma_start(xaug4b[64:65, cbase:cbase + HALF],
                                      src[srow + O:srow + O + 1, :])

            # next layer's per-own-tile bias from the locally staged xx row
            negxx_next = stg.tile([P, NTILES], F32, tag="negxx")
            for t in range(OWN_TILES):
                pt = ps_misc()
                nc.tensor.transpose(pt[0:P, 0:1],
                                    xxrow_sb[:, t * P:(t + 1) * P],
                                    ident[0:1, 0:1])
                nc.scalar.copy(negxx_next[:, t:t + 1], pt[0:P, 0:1])
            negxx_t = negxx_next
            xaug = xaug_next

    # ---- MLP over retained feature-major tiles ----
    mw1 = const.tile([P, 4096], F32)
    nc.sync.dma_start(mw1[:], ins["mw1"][:])
    mc1 = const.tile([P, 8], F32)
    nc.sync.dma_start(mc1[:], ins["mc1"][:])
    mw2 = const.tile([P, 2048], F32)
    nc.sync.dma_start(mw2[:], ins["mw2"][:])
    mc2 = const.tile([P, 2], F32)
    nc.sync.dma_start(mc2[:], ins["mc2"][:])
    mw3 = const.tile([P, 4], F32)
    nc.sync.dma_start(mw3[:], ins["mw3"][:])
    mc3 = const.tile([2, 1], F32)
    nc.sync.dma_start(mc3[:], ins["mc3"][:])

    chunks = [tileA, tileB, tileC, tileD]
    for t in range(OWN_TILES):
        h1 = pdp.tile([P, 1024], F32, tag="pd", name="h1")
        for oc in range(8):
            ps = ps_pd()
            for kc in range(4):
                nc.tensor.matmul(
                    ps[:, 0:P],
                    lhsT=mw1[:, kc * 1024 + oc * P:kc * 1024 + (oc + 1) * P],
                    rhs=chunks[kc][:, t * P:(t + 1) * P],
                    start=(kc == 0), stop=(kc == 3))
            nc.scalar.activation(h1[:, oc * P:(oc + 1) * P], ps[:, 0:P],
                                 AF.Relu, bias=mc1[:, oc:oc + 1], scale=1.0)
        h2 = stg.tile([P, 256], F32, tag="h2")
        for oc in range(2):
            ps = ps_yt()
            for kc in range(8):
                nc.tensor.matmul(
                    ps[:, 0:P],
                    lhsT=mw2[:, kc * 256 + oc * P:kc * 256 + (oc + 1) * P],
                    rhs=h1[:, kc * P:(kc + 1) * P],
                    start=(kc == 0), stop=(kc == 7))
            nc.scalar.activation(h2[:, oc * P:(oc + 1) * P], ps[:, 0:P],
                                 AF.Relu, bias=mc2[:, oc:oc + 1], scale=1.0)
        ps3 = ps_misc()
        for kc in range(2):
            nc.tensor.matmul(ps3[0:2, 0:P], lhsT=mw3[:, kc * 2:(kc + 1) * 2],
                             rhs=h2[:, kc * P:(kc + 1) * P],
                             start=(kc == 0), stop=(kc == 1))
        osb = stg.tile([2, P], F32, tag="osb")
        nc.scalar.activation(osb[:], ps3[0:2, 0:P], AF.Identity,
                             bias=mc3[:], scale=1.0)
        nc.sync.dma_start(
            out_d[t * P:(t + 1) * P, :].rearrange("p c -> c p"), osb[:])


# --------------------------------------------------------------------------
# entry point
# --------------------------------------------------------------------------

_PROGRAM = None


def kernel(**inputs) -> np.ndarray:
    global _PROGRAM
    from concourse import bass_utils

    inputs = {k: np.asarray(v, np.float32) for k, v in inputs.items()}
    B = inputs["x"].shape[0]
    if _PROGRAM is None:
        _PROGRAM = build_program()
    nc = _PROGRAM
    in_maps = make_core_inputs(inputs)
    res = bass_utils.run_bass_kernel_spmd(nc, in_maps, core_ids=list(range(8)))
    outs = [r["out"] for r in res.results]
    full = np.empty((B, N, 2), np.float32)
    for c in range(8):
        b = (c // 2) % B
        h = c % 2
        if SHARD_HALVES:
            full[b, h * HALF:(h + 1) * HALF] = outs[c]
        elif c < B:
            full[c] = outs[c]
    return full


if __name__ == "__main__":
    import reference

    inputs = reference.setup_inputs()
    out = kernel(**{k: np.asarray(v) for k, v in inputs.items()})
    print(out.shape, out.dtype)



# revision 19
# speedup vs baseline: 1.2260x; 1.2260x over previous
"""DGCNN forward on 8 Trainium2 NeuronCores (Bass/Tile).

Contract: kernel(**inputs) takes the FULL unsharded inputs of
reference.setup_inputs() and returns the FULL (4, 4096, 2) output.

Sharding: cores (2b, 2b+1) handle batch b; each core computes half the
rows (2048) of every edge-conv layer (NxN knn + top-20 gather + max) and
of the point-wise MLP. Between layers the pair exchanges its half of the
transposed feature map (plus a -|x|^2/2 row) via a pairwise AllGather.

Math: eval-BN is affine with positive scale s, and ReLU/max commute with
positive affine maps, so each edge-conv collapses to
    out[i] = ReLU( max_{j in knn20(i)} Y[j] + T[i] )
with Y = X @ (s*Wa).T  (gathered by knn index: 20 rows per point via
indirect DMA, max-combined across two 10-row gathers) and
T = X @ (s*(Wb-Wa)).T + (s*(b-m)+beta)  from a small matmul.
knn ranking uses pd = inner - xx_i/2 - xx_j/2 (= reference pd / 2, same
ordering), computed on the PE; top-20 selection runs on the Vector
engine (max8 / max_index / match_replace rounds).
"""

import sys

sys.path.insert(0, "/opt/trn_rl_repo")

import math
import os
from contextlib import ExitStack

import numpy as np

import concourse.bass as bass
import concourse.tile as tile
from concourse import bacc, mybir
from concourse.masks import make_identity

EPS = 1e-5
K = 20
N = 4096
P = 128
NTILES = N // P            # 32 point tiles per batch
NEG = -3.0e38
F32 = mybir.dt.float32
U32 = mybir.dt.uint32
AF = mybir.ActivationFunctionType
OP = mybir.AluOpType

# (C_in, O_out) per edge-conv layer
DIMS = [(3, 64), (64, 64), (64, 128), (128, 256)]

# False: every core computes its full batch (no collectives) - debug mode.
SHARD_HALVES = os.environ.get("DGCNN_SHARD", "1") == "1"
TOPK_MODE = os.environ.get("DGCNN_TOPK", "flat")
USE_F32R = os.environ.get("DGCNN_F32R", "0") == "1"
SCAN16 = os.environ.get("DGCNN_SCAN16", "1") == "1"
GATHER1 = os.environ.get("DGCNN_GATHER1", "1") == "1"
YF16 = os.environ.get("DGCNN_YF16", "1") == "1"
REDGP = os.environ.get("DGCNN_REDGP", "1") == "1"
F16 = mybir.dt.float16
F32R = mybir.dt.float32r
SCAN_DT = F16 if SCAN16 else mybir.dt.float32
SCAN_NEG = -60000.0 if SCAN16 else NEG
Y_DT = F16 if YF16 else mybir.dt.float32

HALF = N // 2 if SHARD_HALVES else N
OWN_TILES = HALF // P      # 16 (32 in debug mode)


# --------------------------------------------------------------------------
# host-side weight folding
# --------------------------------------------------------------------------

def _fold(inputs):
    d = {}
    for l, (C, O) in enumerate(DIMS, 1):
        w = inputs[f"cw{l}"]            # (O, 2C)
        b = inputs[f"cb{l}"]
        g, be = inputs[f"g{l}"], inputs[f"b{l}"]
        m, v = inputs[f"m{l}"], inputs[f"v{l}"]
        s = g / np.sqrt(v + EPS)
        Wa, Wb = w[:, :C], w[:, C:]
        WaP = (s[:, None] * Wa).T.astype(np.float32)        # (C, O)
        W2P = (s[:, None] * (Wb - Wa)).T.astype(np.float32)  # (C, O)
        cP = (s * (b - m) + be).astype(np.float32)           # (O,)
        d[f"wa{l}"] = np.ascontiguousarray(WaP.astype(np.float16))
        if l < 4:
            d[f"w2{l}"] = np.ascontiguousarray(
                np.concatenate([W2P, cP[None, :]], 0).astype(np.float16))
        else:
            d["w24a"] = np.ascontiguousarray(W2P[0:64].astype(np.float16))
            d["w24b"] = np.ascontiguousarray(
                np.concatenate([W2P[64:128], cP[None, :]],
                               0).astype(np.float16))        # (65, 256)

    def fold_lin(w, b, g, be, m, v):
        s = g / np.sqrt(v + EPS)
        return ((s[:, None] * w).T.astype(np.float32),
                (s * (b - m) + be).astype(np.float32))

    L1, c1 = fold_lin(inputs["l1w"], inputs["l1b"], inputs["f1g"],
                      inputs["f1b"], inputs["f1m"], inputs["f1v"])
    L2, c2 = fold_lin(inputs["l2w"], inputs["l2b"], inputs["f2g"],
                      inputs["f2b"], inputs["f2m"], inputs["f2v"])
    L3 = inputs["l3w"].T.astype(np.float32)                  # (256, 2)
    c3 = inputs["l3b"].astype(np.float32)

    d["mw1"] = np.ascontiguousarray(
        L1.reshape(4, 128, 1024).transpose(1, 0, 2).reshape(128, 4096)
        .astype(np.float16))
    d["mc1"] = np.ascontiguousarray(c1.reshape(8, 128).T)    # [128, 8]
    d["mw2"] = np.ascontiguousarray(
        L2.reshape(8, 128, 256).transpose(1, 0, 2).reshape(128, 2048)
        .astype(np.float16))
    d["mc2"] = np.ascontiguousarray(c2.reshape(2, 128).T)    # [128, 2]
    d["mw3"] = np.ascontiguousarray(
        L3.reshape(2, 128, 2).transpose(1, 0, 2).reshape(128, 4)
        .astype(np.float16))
    d["mc3"] = np.ascontiguousarray(c3[:, None])             # [2, 1]
    return d


def make_core_inputs(inputs):
    """Per-core input dicts (len 8). Core c -> batch c//2, half c%2."""
    shared = _fold(inputs)
    x = np.asarray(inputs["x"], np.float32)          # (B, 4096, 3)
    B = x.shape[0]
    per_core = []
    for c in range(8):
        b = (c // 2) % B if SHARD_HALVES else c % B
        h = c % 2 if SHARD_HALVES else 0
        xb = x[b]
        xx = (xb * xb).sum(-1)
        xaug = np.concatenate([xb.T, -0.5 * xx[None, :]], 0).astype(np.float32)
        d = dict(shared)
        d["xaug1"] = np.ascontiguousarray(xaug.astype(np.float16))   # (4, 4096)
        d["xown1"] = np.ascontiguousarray(
            xaug[:, h * HALF:(h + 1) * HALF].astype(np.float16))
        d["negxx1"] = np.ascontiguousarray(
            xaug[3, h * HALF:(h + 1) * HALF].reshape(HALF // 128, 128).T
            .astype(np.float32))
        per_core.append(d)
    return per_core


# --------------------------------------------------------------------------
# device program
# --------------------------------------------------------------------------

def build_program():
    nc = bacc.Bacc(None)
    ins = {}

    def einp(name, shape, dt=F16):
        ins[name] = nc.dram_tensor(name, shape, dt, kind="ExternalInput")
        return ins[name]

    einp("xaug1", [4, N])
    einp("xown1", [4, HALF])
    einp("negxx1", [128, HALF // 128], F32)
    for l, (C, O) in enumerate(DIMS, 1):
        einp(f"wa{l}", [C, O])
        if l < 4:
            einp(f"w2{l}", [C + 1, O])
    einp("w24a", [64, 256])
    einp("w24b", [65, 256])
    einp("mw1", [128, 4096])
    einp("mc1", [128, 8], F32)
    einp("mw2", [128, 2048])
    einp("mc2", [128, 2], F32)
    einp("mw3", [128, 4])
    einp("mc3", [2, 1], F32)

    out_d = nc.dram_tensor("out", [HALF, 2], F32, kind="ExternalOutput")

    with tile.TileContext(nc) as tc:
        with ExitStack() as ctx:
            _build_tile_graph(ctx, tc, ins, out_d)

    nc.compile()
    return nc


def _build_tile_graph(ctx, tc, ins, out_d):
    nc = tc.nc

    nb = 2 if SHARD_HALVES else 1
    const = ctx.enter_context(tc.tile_pool(name="const", bufs=1))
    big = ctx.enter_context(tc.tile_pool(name="big", bufs=1))
    pdp = ctx.enter_context(tc.tile_pool(name="pdp", bufs=3 if SHARD_HALVES else 1))
    stg = ctx.enter_context(tc.tile_pool(name="stg", bufs=nb))
    gat = ctx.enter_context(tc.tile_pool(name="gat", bufs=1))
    psum = ctx.enter_context(tc.tile_pool(name="psum", bufs=2, space="PSUM"))
    dram = ctx.enter_context(tc.tile_pool(name="dram", bufs=1, space="DRAM"))

    def ps_pd():
        return psum.tile([P, 512], F32, tag="pspd", name="pspd")

    def ps_yt():
        return psum.tile([P, 256], F32, tag="psY", name="psY")

    def ps_misc():
        return psum.tile([P, 512], F32, tag="psmisc", name="psmisc")

    ident = const.tile([P, P], F32)
    make_identity(nc, ident[:])

    wa_sb, w2_sb = {}, {}
    for l, (C, O) in enumerate(DIMS, 1):
        wa_sb[l] = const.tile([C, O], F16, tag=f"wa{l}", name=f"wa{l}sb")
        nc.sync.dma_start(wa_sb[l][:], ins[f"wa{l}"][:])
        if l < 4:
            w2_sb[l] = const.tile([C + 1, O], F16, tag=f"w2{l}", name=f"w2{l}sb")
            nc.sync.dma_start(w2_sb[l][:], ins[f"w2{l}"][:])
    w24a = const.tile([64, 256], F16)
    nc.sync.dma_start(w24a[:], ins["w24a"][:])
    w24b = const.tile([65, 256], F16)
    nc.sync.dma_start(w24b[:], ins["w24b"][:])

    # feature-major inputs: xaug rows 0..C-1 = x^T, row C = -|x|^2/2
    xaug_tag = {1: "xaug_a", 2: "xaug_b", 3: "xaug_a", 4: "xaug_b"}
    xaug = big.tile([P, N], F16, tag=xaug_tag[1])
    nc.sync.dma_start(xaug[0:4, :], ins["xaug1"][:])
    xown1 = big.tile([P, HALF], F16, tag="ownD")
    nc.sync.dma_start(xown1[0:4, :], ins["xown1"][:])

    # retained own-half feature-major activations (MLP k-chunks)
    tileA = big.tile([P, HALF], F16, tag="mlpA")  # x1T (rows 0:64) | x2T (64:128)
    tileB = big.tile([P, HALF], F16, tag="mlpB")  # x3T
    tileC = big.tile([P, HALF], F16, tag="mlpC")  # x4T rows 0:128
    tileD = big.tile([P, HALF], F16, tag="ownD")  # x4T rows 128:256 (reuses xown1)
    # (tile, row_offset) holding each layer's own-half input, feature-major
    xown_map = {1: (xown1, 0), 2: (tileA, 0), 3: (tileA, 64), 4: (tileB, 0)}

    negxx_t = None        # [P, OWN_TILES] per-tile -xx/2 bias (current layer)
    xaug4b = None

    for l, (C, O) in enumerate(DIMS, 1):
        # ---- stage A: Y' for the whole batch -> DRAM ----
        ydram = dram.tile([N, O], Y_DT, tag=f"yd{l}")
        for j in range(NTILES):
            ps = ps_yt()
            nc.tensor.matmul(ps[:, 0:O], lhsT=xaug[0:C, j * P:(j + 1) * P],
                             rhs=wa_sb[l][:], start=True, stop=True)
            ysb = stg.tile([P, 256], Y_DT, tag="ysb")
            nc.scalar.copy(ysb[:, 0:O], ps[:, 0:O])
            nc.sync.dma_start(ydram[j * P:(j + 1) * P, :], ysb[:, 0:O])

        if l == 1:
            negxx_t = stg.tile([P, NTILES], F32, tag="negxx")
            nc.sync.dma_start(negxx_t[:, 0:OWN_TILES], ins["negxx1"][:])

        if l < 4:
            eg_in = dram.tile([O + 1, HALF], F16, tag=f"egin{l}")
            if SHARD_HALVES:
                eg_out = dram.tile([2 * (O + 1), HALF], F16, tag=f"egout{l}")
            xxrow_sb = big.tile([1, HALF], F16, tag="xxrow")
            negxx_next = stg.tile([P, NTILES], F32, tag="negxx")

        own_src, own_ro = xown_map[l]

        # ---- stage B: per own row-tile ----
        for t in range(OWN_TILES):
            # lhsT staging: rows 0..C-1 = x^T own slice, row C = ones
            lt = stg.tile([P, P], F16, tag="lhsT")
            if C < 32:
                nc.gpsimd.memset(lt[0:32, :], 1.0)
            elif l < 4:
                nc.gpsimd.memset(lt[C:C + 1, :], 1.0)
            nc.scalar.copy(lt[0:C if l < 4 else 64, :],
                           own_src[own_ro:own_ro + (C if l < 4 else 64),
                                   t * P:(t + 1) * P])
            kk = C + 1 if l < 4 else 64
            if l == 4:
                # K-split: second chunk carries rows 64:128 plus a ones row
                # (for cP / the -xx_j/2 row on the rhs side)
                lt4b = stg.tile([P, P], F16, tag="lhsT4b")
                nc.gpsimd.memset(lt4b[64:65, :], 1.0)
                nc.scalar.copy(lt4b[0:64, :],
                               own_src[64:128, t * P:(t + 1) * P])

            # T' = x @ W2P + cP  (cP via the ones row)
            psT = ps_yt()
            if l < 4:
                nc.tensor.matmul(psT[:, 0:O], lhsT=lt[0:kk, :], rhs=w2_sb[l][:],
                                 start=True, stop=True)
            else:
                nc.tensor.matmul(psT[:, 0:O], lhsT=lt[0:64, :], rhs=w24a[:],
                                 start=True, stop=False)
                nc.tensor.matmul(psT[:, 0:O], lhsT=lt4b[0:65, :], rhs=w24b[:],
                                 start=False, stop=True)
            tsb = stg.tile([P, 256], F32, tag="tsb")
            nc.scalar.copy(tsb[:, 0:O], psT[:, 0:O])

            # pd row-block (f32r matmul: 1 cycle/row vs 4 for fp32; rank-only)
            def mdt(ap):
                return ap.bitcast(F32R) if USE_F32R else ap
            pd = pdp.tile([P, N], SCAN_DT, tag="pd")
            for cc in range(8):
                ppd = ps_pd()
                sl = slice(cc * 512, (cc + 1) * 512)
                if l < 4:
                    nc.tensor.matmul(ppd[:], lhsT=mdt(lt[0:kk, :]),
                                     rhs=mdt(xaug[0:kk, sl]),
                                     start=True, stop=True)
                else:
                    nc.tensor.matmul(ppd[:], lhsT=mdt(lt[0:64, :]),
                                     rhs=mdt(xaug[0:64, sl]),
                                     start=True, stop=False)
                    nc.tensor.matmul(ppd[:], lhsT=mdt(lt4b[0:65, :]),
                                     rhs=mdt(xaug4b[0:65, sl]),
                                     start=False, stop=True)
                nc.scalar.activation(pd[:, sl], ppd[:], AF.Identity,
                                     bias=negxx_t[:, t:t + 1], scale=1.0)

            # top-20 neighbour indices
            idxs = stg.tile([P, 24], U32, tag="idxs")
            vals = stg.tile([P, 24], SCAN_DT, tag="vals")
            for r in range(3):
                v8 = vals[:, r * 8:r * 8 + 8]
                i8 = idxs[:, r * 8:r * 8 + 8]
                nc.vector.max(out=v8, in_=pd[:])
                nc.vector.max_index(out=i8, in_max=v8, in_values=pd[:])
                if r < 2:
                    nc.vector.match_replace(out=pd[:], in_to_replace=v8,
                                            in_values=pd[:], imm_value=SCAN_NEG)

            # gather the 20 neighbour Y' rows, then max over k.  The swdge
            # walks the offset AP in ravel order, so a [128,20] offset AP
            # fetches all 20 rows per point in ONE ~1us instruction.
            G = gat.tile([P, 20 * O], Y_DT, tag="G")
            if GATHER1:
                nc.gpsimd.indirect_dma_start(
                    out=G[:, 0:20 * O], out_offset=None, in_=ydram[:],
                    in_offset=bass.IndirectOffsetOnAxis(ap=idxs[:, 0:20],
                                                        axis=0))
            else:
                for k in range(20):
                    nc.gpsimd.indirect_dma_start(
                        out=G[:, k * O:(k + 1) * O], out_offset=None,
                        in_=ydram[:],
                        in_offset=bass.IndirectOffsetOnAxis(ap=idxs[:, k:k + 1],
                                                            axis=0))
            Mx = stg.tile([P, 256], F32, tag="Mx")
            if REDGP:
                # packed fp16 max tree 20 -> 10 -> 5 -> (4+1) -> 1 chunks
                # of O on the DVE (2x mode; walrus rejects max on Pool)
                mt = gat.tile([P, 18 * 256], Y_DT, tag="mt")
                nc.vector.tensor_max(mt[:, 0:10 * O], G[:, 0:10 * O],
                                     G[:, 10 * O:20 * O])
                nc.vector.tensor_max(mt[:, 10 * O:15 * O], mt[:, 0:5 * O],
                                     mt[:, 5 * O:10 * O])
                nc.vector.tensor_max(mt[:, 15 * O:17 * O],
                                     mt[:, 10 * O:12 * O],
                                     mt[:, 12 * O:14 * O])
                nc.vector.tensor_max(mt[:, 17 * O:18 * O],
                                     mt[:, 15 * O:16 * O],
                                     mt[:, 16 * O:17 * O])
                nc.vector.tensor_max(Mx[:, 0:O], mt[:, 17 * O:18 * O],
                                     mt[:, 14 * O:15 * O])
            else:
                nc.vector.tensor_reduce(
                    out=Mx[:, 0:O],
                    in_=G[:].rearrange("p (s o) -> p o s", s=20, o=O),
                    axis=mybir.AxisListType.X, op=OP.max)

            # out = ReLU(Mx + T')
            pre = stg.tile([P, 256], F32, tag="pre")
            nc.gpsimd.tensor_add(pre[:, 0:O], Mx[:, 0:O], tsb[:, 0:O])
            opm = stg.tile([P, 256], F32, tag="opm")
            nc.scalar.activation(opm[:, 0:O], pre[:, 0:O], AF.Relu)

            # transpose to feature-major, retain own half, stage exchange
            if l < 4:
                ptr = ps_misc()
                nc.tensor.transpose(ptr[0:O, 0:P], opm[:, 0:O], ident[:])
                if l == 1:
                    dst = tileA[0:64, t * P:(t + 1) * P]
                elif l == 2:
                    dst = tileA[64:128, t * P:(t + 1) * P]
                else:
                    dst = tileB[0:128, t * P:(t + 1) * P]
                nc.scalar.copy(dst, ptr[0:O, 0:P])
                nc.sync.dma_start(eg_in[0:O, t * P:(t + 1) * P], dst)

                # -|x_new|^2/2 per point
                sq = stg.tile([P, 256], F32, tag="sq")
                xxc = stg.tile([P, 2], F32, tag="xxc")
                nc.scalar.activation(sq[:, 0:O], opm[:, 0:O], AF.Square,
                                     scale=float(math.sqrt(0.5)),
                                     accum_out=xxc[:, 0:1])
                nc.scalar.activation(xxc[:, 1:2], xxc[:, 0:1], AF.Identity,
                                     scale=-1.0)
                # next layer's per-own-tile bias, partition-major (no
                # transpose needed)
                nc.scalar.copy(negxx_next[:, t:t + 1], xxc[:, 1:2])
                pxr = ps_misc()
                nc.tensor.transpose(pxr[0:1, 0:P], xxc[:, 1:2], ident[:])
                nc.scalar.copy(xxrow_sb[:, t * P:(t + 1) * P], pxr[0:1, 0:P])
            else:
                ptr = ps_misc()
                nc.tensor.transpose(ptr[0:P, 0:P], opm[:, 0:128], ident[:])
                nc.scalar.copy(tileC[:, t * P:(t + 1) * P], ptr[0:P, 0:P])
                ptr2 = ps_misc()
                nc.tensor.transpose(ptr2[0:P, 0:P], opm[:, 128:256], ident[:])
                nc.scalar.copy(tileD[:, t * P:(t + 1) * P], ptr2[0:P, 0:P])

        # ---- stage C: exchange halves, assemble next layer's inputs ----
        if l < 4:
            nc.sync.dma_start(eg_in[O:O + 1, :], xxrow_sb[:])
            if SHARD_HALVES:
                nc.gpsimd.collective_compute(
                    "AllGather", OP.bypass,
                    replica_groups=[[0, 1], [2, 3], [4, 5], [6, 7]],
                    ins=[eg_in.opt()], outs=[eg_out.opt()])
                src, blocks = eg_out, [(0, 0), (O + 1, HALF)]
            else:
                src, blocks = eg_in, [(0, 0)]

            xaug_next = big.tile([P, N], F16, tag=xaug_tag[l + 1])
            if l < 3:
                for (srow, cbase) in blocks:
                    nc.sync.dma_start(xaug_next[0:O, cbase:cbase + HALF],
                                      src[srow:srow + O, :])
                    nc.sync.dma_start(xaug_next[O:O + 1, cbase:cbase + HALF],
                                      src[srow + O:srow + O + 1, :])
            else:
                # layer-4 rhs K-chunks: xaug4 rows 0:64 (via xaug_next) and
                # xaug4b = rows 64:128 plus the -xx/2 row
                xaug4b = big.tile([P, N], F16, tag="xaug_a")
                for (srow, cbase) in blocks:
                    nc.sync.dma_start(xaug_next[0:O, cbase:cbase + HALF],
                                      src[srow:srow + O, :])
                    nc.sync.dma_start(xaug4b[0:64, cbase:cbase + HALF],
                                      src[srow + 64:srow + 128, :])
                    nc.sync.dma_start(xaug4b[64:65, cbase:cbase + HALF],
                                      src[srow + O:srow + O + 1, :])

            negxx_t = negxx_next
            xaug = xaug_next

    # ---- MLP over retained feature-major tiles ----
    mw1 = const.tile([P, 4096], F16)
    nc.sync.dma_start(mw1[:], ins["mw1"][:])
    mc1 = const.tile([P, 8], F32)
    nc.sync.dma_start(mc1[:], ins["mc1"][:])
    mw2 = const.tile([P, 2048], F16)
    nc.sync.dma_start(mw2[:], ins["mw2"][:])
    mc2 = const.tile([P, 2], F32)
    nc.sync.dma_start(mc2[:], ins["mc2"][:])
    mw3 = const.tile([P, 4], F16)
    nc.sync.dma_start(mw3[:], ins["mw3"][:])
    mc3 = const.tile([2, 1], F32)
    nc.sync.dma_start(mc3[:], ins["mc3"][:])

    chunks = [tileA, tileB, tileC, tileD]
    for t in range(OWN_TILES):
        h1 = pdp.tile([P, 1024], F16, tag="pd", name="h1")
        for oc in range(8):
            ps = ps_pd()
            for kc in range(4):
                nc.tensor.matmul(
                    ps[:, 0:P],
                    lhsT=mw1[:, kc * 1024 + oc * P:kc * 1024 + (oc + 1) * P],
                    rhs=chunks[kc][:, t * P:(t + 1) * P],
                    start=(kc == 0), stop=(kc == 3))
            nc.scalar.activation(h1[:, oc * P:(oc + 1) * P], ps[:, 0:P],
                                 AF.Relu, bias=mc1[:, oc:oc + 1], scale=1.0)
        h2 = stg.tile([P, 256], F16, tag="h2")
        for oc in range(2):
            ps = ps_yt()
            for kc in range(8):
                nc.tensor.matmul(
                    ps[:, 0:P],
                    lhsT=mw2[:, kc * 256 + oc * P:kc * 256 + (oc + 1) * P],
                    rhs=h1[:, kc * P:(kc + 1) * P],
                    start=(kc == 0), stop=(kc == 7))
            nc.scalar.activation(h2[:, oc * P:(oc + 1) * P], ps[:, 0:P],
                                 AF.Relu, bias=mc2[:, oc:oc + 1], scale=1.0)
        ps3 = ps_misc()
        for kc in range(2):
            nc.tensor.matmul(ps3[0:2, 0:P], lhsT=mw3[:, kc * 2:(kc + 1) * 2],
                             rhs=h2[:, kc * P:(kc + 1) * P],
                             start=(kc == 0), stop=(kc == 1))
        osb = stg.tile([2, P], F32, tag="osb")
        nc.scalar.activation(osb[:], ps3[0:2, 0:P], AF.Identity,
                             bias=mc3[:], scale=1.0)
        nc.sync.dma_start(
            out_d[t * P:(t + 1) * P, :].rearrange("p c -> c p"), osb[:])


# --------------------------------------------------------------------------
# entry point
# --------------------------------------------------------------------------

_PROGRAM = None


def kernel(**inputs) -> np.ndarray:
    global _PROGRAM
    from concourse import bass_utils

    inputs = {k: np.asarray(v, np.float32) for k, v in inputs.items()}
    B = inputs["x"].shape[0]
    if _PROGRAM is None:
        _PROGRAM = build_program()
    nc = _PROGRAM
    in_maps = make_core_inputs(inputs)
    res = bass_utils.run_bass_kernel_spmd(nc, in_maps, core_ids=list(range(8)))
    outs = [r["out"] for r in res.results]
    full = np.empty((B, N, 2), np.float32)
    for c in range(8):
        b = (c // 2) % B
        h = c % 2
        if SHARD_HALVES:
            full[b, h * HALF:(h + 1) * HALF] = outs[c]
        elif c < B:
            full[c] = outs[c]
    return full


if __name__ == "__main__":
    import reference

    inputs = reference.setup_inputs()
    out = kernel(**{k: np.asarray(v) for k, v in inputs.items()})
    print(out.shape, out.dtype)



# revision 22
# speedup vs baseline: 1.3296x; 1.0845x over previous
"""DGCNN forward on 8 Trainium2 NeuronCores (Bass/Tile).

Contract: kernel(**inputs) takes the FULL unsharded inputs of
reference.setup_inputs() and returns the FULL (4, 4096, 2) output.

Sharding: cores (2b, 2b+1) handle batch b; each core computes half the
rows (2048) of every edge-conv layer (NxN knn + top-20 gather + max) and
of the point-wise MLP. Between layers the pair exchanges its half of the
transposed feature map (plus a -|x|^2/2 row) via a pairwise AllGather.

Math: eval-BN is affine with positive scale s, and ReLU/max commute with
positive affine maps, so each edge-conv collapses to
    out[i] = ReLU( max_{j in knn20(i)} Y[j] + T[i] )
with Y = X @ (s*Wa).T  (gathered by knn index: 20 rows per point via
indirect DMA, max-combined across two 10-row gathers) and
T = X @ (s*(Wb-Wa)).T + (s*(b-m)+beta)  from a small matmul.
knn ranking uses pd = inner - xx_i/2 - xx_j/2 (= reference pd / 2, same
ordering), computed on the PE; top-20 selection runs on the Vector
engine (max8 / max_index / match_replace rounds).
"""

import sys

sys.path.insert(0, "/opt/trn_rl_repo")

import math
import os
from contextlib import ExitStack

import numpy as np

import concourse.bass as bass
import concourse.tile as tile
from concourse import bacc, mybir
from concourse.masks import make_identity

EPS = 1e-5
K = 20
N = 4096
P = 128
NTILES = N // P            # 32 point tiles per batch
NEG = -3.0e38
F32 = mybir.dt.float32
U32 = mybir.dt.uint32
AF = mybir.ActivationFunctionType
OP = mybir.AluOpType

# (C_in, O_out) per edge-conv layer
DIMS = [(3, 64), (64, 64), (64, 128), (128, 256)]

# False: every core computes its full batch (no collectives) - debug mode.
SHARD_HALVES = os.environ.get("DGCNN_SHARD", "1") == "1"
TOPK_MODE = os.environ.get("DGCNN_TOPK", "flat")
USE_F32R = os.environ.get("DGCNN_F32R", "0") == "1"
SCAN16 = os.environ.get("DGCNN_SCAN16", "0") == "1"
HIER = os.environ.get("DGCNN_HIER", "1") == "1"
# multi-offset single-instruction gather: CoreSim supports it but real HW
# wedges (NRT_EXEC_UNIT_UNRECOVERABLE) — one dynamic offset per partition
# per instruction is a hardware limit. Keep off.
GATHER1 = os.environ.get("DGCNN_GATHER1", "0") == "1"
YF16 = os.environ.get("DGCNN_YF16", "1") == "1"
REDGP = os.environ.get("DGCNN_REDGP", "1") == "1"
F16 = mybir.dt.float16
F32R = mybir.dt.float32r
SCAN_DT = F16 if SCAN16 else mybir.dt.float32
SCAN_NEG = -60000.0 if SCAN16 else NEG
Y_DT = F16 if YF16 else mybir.dt.float32

HALF = N // 2 if SHARD_HALVES else N
OWN_TILES = HALF // P      # 16 (32 in debug mode)


# --------------------------------------------------------------------------
# host-side weight folding
# --------------------------------------------------------------------------

def _fold(inputs):
    d = {}
    for l, (C, O) in enumerate(DIMS, 1):
        w = inputs[f"cw{l}"]            # (O, 2C)
        b = inputs[f"cb{l}"]
        g, be = inputs[f"g{l}"], inputs[f"b{l}"]
        m, v = inputs[f"m{l}"], inputs[f"v{l}"]
        s = g / np.sqrt(v + EPS)
        Wa, Wb = w[:, :C], w[:, C:]
        WaP = (s[:, None] * Wa).T.astype(np.float32)        # (C, O)
        W2P = (s[:, None] * (Wb - Wa)).T.astype(np.float32)  # (C, O)
        cP = (s * (b - m) + be).astype(np.float32)           # (O,)
        d[f"wa{l}"] = np.ascontiguousarray(WaP.astype(np.float16))
        if l < 4:
            d[f"w2{l}"] = np.ascontiguousarray(
                np.concatenate([W2P, cP[None, :]], 0).astype(np.float16))
        else:
            d["w24a"] = np.ascontiguousarray(W2P[0:64].astype(np.float16))
            d["w24b"] = np.ascontiguousarray(
                np.concatenate([W2P[64:128], cP[None, :]],
                               0).astype(np.float16))        # (65, 256)

    def fold_lin(w, b, g, be, m, v):
        s = g / np.sqrt(v + EPS)
        return ((s[:, None] * w).T.astype(np.float32),
                (s * (b - m) + be).astype(np.float32))

    L1, c1 = fold_lin(inputs["l1w"], inputs["l1b"], inputs["f1g"],
                      inputs["f1b"], inputs["f1m"], inputs["f1v"])
    L2, c2 = fold_lin(inputs["l2w"], inputs["l2b"], inputs["f2g"],
                      inputs["f2b"], inputs["f2m"], inputs["f2v"])
    L3 = inputs["l3w"].T.astype(np.float32)                  # (256, 2)
    c3 = inputs["l3b"].astype(np.float32)

    d["mw1"] = np.ascontiguousarray(
        L1.reshape(4, 128, 1024).transpose(1, 0, 2).reshape(128, 4096)
        .astype(np.float16))
    d["mc1"] = np.ascontiguousarray(c1.reshape(8, 128).T)    # [128, 8]
    d["mw2"] = np.ascontiguousarray(
        L2.reshape(8, 128, 256).transpose(1, 0, 2).reshape(128, 2048)
        .astype(np.float16))
    d["mc2"] = np.ascontiguousarray(c2.reshape(2, 128).T)    # [128, 2]
    d["mw3"] = np.ascontiguousarray(
        L3.reshape(2, 128, 2).transpose(1, 0, 2).reshape(128, 4)
        .astype(np.float16))
    d["mc3"] = np.ascontiguousarray(c3[:, None])             # [2, 1]
    return d


def make_core_inputs(inputs):
    """Per-core input dicts (len 8). Core c -> batch c//2, half c%2."""
    shared = _fold(inputs)
    x = np.asarray(inputs["x"], np.float32)          # (B, 4096, 3)
    B = x.shape[0]
    per_core = []
    for c in range(8):
        b = (c // 2) % B if SHARD_HALVES else c % B
        h = c % 2 if SHARD_HALVES else 0
        xb = x[b]
        xx = (xb * xb).sum(-1)
        xaug = np.concatenate([xb.T, -0.5 * xx[None, :]], 0).astype(np.float32)
        d = dict(shared)
        d["xaug1"] = np.ascontiguousarray(xaug.astype(np.float16))   # (4, 4096)
        d["xown1"] = np.ascontiguousarray(
            xaug[:, h * HALF:(h + 1) * HALF].astype(np.float16))
        d["negxx1"] = np.ascontiguousarray(
            xaug[3, h * HALF:(h + 1) * HALF].reshape(HALF // 128, 128).T
            .astype(np.float32))
        per_core.append(d)
    return per_core


# --------------------------------------------------------------------------
# device program
# --------------------------------------------------------------------------

def build_program():
    nc = bacc.Bacc(None)
    ins = {}

    def einp(name, shape, dt=F16):
        ins[name] = nc.dram_tensor(name, shape, dt, kind="ExternalInput")
        return ins[name]

    einp("xaug1", [4, N])
    einp("xown1", [4, HALF])
    einp("negxx1", [128, HALF // 128], F32)
    for l, (C, O) in enumerate(DIMS, 1):
        einp(f"wa{l}", [C, O])
        if l < 4:
            einp(f"w2{l}", [C + 1, O])
    einp("w24a", [64, 256])
    einp("w24b", [65, 256])
    einp("mw1", [128, 4096])
    einp("mc1", [128, 8], F32)
    einp("mw2", [128, 2048])
    einp("mc2", [128, 2], F32)
    einp("mw3", [128, 4])
    einp("mc3", [2, 1], F32)

    out_d = nc.dram_tensor("out", [HALF, 2], F32, kind="ExternalOutput")

    with tile.TileContext(nc) as tc:
        with ExitStack() as ctx:
            _build_tile_graph(ctx, tc, ins, out_d)

    nc.compile()
    return nc


def _build_tile_graph(ctx, tc, ins, out_d):
    nc = tc.nc

    nb = 2 if SHARD_HALVES else 1
    const = ctx.enter_context(tc.tile_pool(name="const", bufs=1))
    big = ctx.enter_context(tc.tile_pool(name="big", bufs=1))
    pdp = ctx.enter_context(tc.tile_pool(name="pdp", bufs=3 if SHARD_HALVES else 1))
    stg = ctx.enter_context(tc.tile_pool(name="stg", bufs=nb))
    gat = ctx.enter_context(tc.tile_pool(name="gat", bufs=1))
    psum = ctx.enter_context(tc.tile_pool(name="psum", bufs=2, space="PSUM"))
    dram = ctx.enter_context(tc.tile_pool(name="dram", bufs=1, space="DRAM"))

    def ps_pd():
        return psum.tile([P, 512], F32, tag="pspd", name="pspd")

    def ps_yt():
        return psum.tile([P, 256], F32, tag="psY", name="psY")

    def ps_misc():
        return psum.tile([P, 512], F32, tag="psmisc", name="psmisc")

    ident = const.tile([P, P], F32)
    make_identity(nc, ident[:])

    wa_sb, w2_sb = {}, {}
    for l, (C, O) in enumerate(DIMS, 1):
        wa_sb[l] = const.tile([C, O], F16, tag=f"wa{l}", name=f"wa{l}sb")
        nc.sync.dma_start(wa_sb[l][:], ins[f"wa{l}"][:])
        if l < 4:
            w2_sb[l] = const.tile([C + 1, O], F16, tag=f"w2{l}", name=f"w2{l}sb")
            nc.sync.dma_start(w2_sb[l][:], ins[f"w2{l}"][:])
    w24a = const.tile([64, 256], F16)
    nc.sync.dma_start(w24a[:], ins["w24a"][:])
    w24b = const.tile([65, 256], F16)
    nc.sync.dma_start(w24b[:], ins["w24b"][:])

    # feature-major inputs: xaug rows 0..C-1 = x^T, row C = -|x|^2/2
    xaug_tag = {1: "xaug_a", 2: "xaug_b", 3: "xaug_a", 4: "xaug_b"}
    xaug = big.tile([P, N], F16, tag=xaug_tag[1])
    nc.sync.dma_start(xaug[0:4, :], ins["xaug1"][:])
    xown1 = big.tile([P, HALF], F16, tag="ownD")
    nc.sync.dma_start(xown1[0:4, :], ins["xown1"][:])

    # retained own-half feature-major activations (MLP k-chunks)
    tileA = big.tile([P, HALF], F16, tag="mlpA")  # x1T (rows 0:64) | x2T (64:128)
    tileB = big.tile([P, HALF], F16, tag="mlpB")  # x3T
    tileC = big.tile([P, HALF], F16, tag="mlpC")  # x4T rows 0:128
    tileD = big.tile([P, HALF], F16, tag="ownD")  # x4T rows 128:256 (reuses xown1)
    # (tile, row_offset) holding each layer's own-half input, feature-major
    xown_map = {1: (xown1, 0), 2: (tileA, 0), 3: (tileA, 64), 4: (tileB, 0)}

    negxx_t = None        # [P, OWN_TILES] per-tile -xx/2 bias (current layer)
    xaug4b = None

    for l, (C, O) in enumerate(DIMS, 1):
        # ---- stage A: Y' for the whole batch -> DRAM ----
        ydram = dram.tile([N, O], Y_DT, tag=f"yd{l}")
        for j in range(NTILES):
            ps = ps_yt()
            nc.tensor.matmul(ps[:, 0:O], lhsT=xaug[0:C, j * P:(j + 1) * P],
                             rhs=wa_sb[l][:], start=True, stop=True)
            ysb = stg.tile([P, 256], Y_DT, tag="ysb")
            nc.scalar.copy(ysb[:, 0:O], ps[:, 0:O])
            nc.sync.dma_start(ydram[j * P:(j + 1) * P, :], ysb[:, 0:O])

        if l == 1:
            negxx_t = stg.tile([P, NTILES], F32, tag="negxx")
            nc.sync.dma_start(negxx_t[:, 0:OWN_TILES], ins["negxx1"][:])

        if l < 4:
            eg_in = dram.tile([O + 1, HALF], F16, tag=f"egin{l}")
            if SHARD_HALVES:
                eg_out = dram.tile([2 * (O + 1), HALF], F16, tag=f"egout{l}")
            xxrow_sb = big.tile([1, HALF], F16, tag="xxrow")
            negxx_next = stg.tile([P, NTILES], F32, tag="negxx")

        own_src, own_ro = xown_map[l]

        # ---- stage B: per own row-tile ----
        for t in range(OWN_TILES):
            # lhsT staging: rows 0..C-1 = x^T own slice, row C = ones
            lt = stg.tile([P, P], F16, tag="lhsT")
            if C < 32:
                nc.gpsimd.memset(lt[0:32, :], 1.0)
            elif l < 4:
                nc.gpsimd.memset(lt[C:C + 1, :], 1.0)
            nc.scalar.copy(lt[0:C if l < 4 else 64, :],
                           own_src[own_ro:own_ro + (C if l < 4 else 64),
                                   t * P:(t + 1) * P])
            kk = C + 1 if l < 4 else 64
            if l == 4:
                # K-split: second chunk carries rows 64:128 plus a ones row
                # (for cP / the -xx_j/2 row on the rhs side)
                lt4b = stg.tile([P, P], F16, tag="lhsT4b")
                nc.gpsimd.memset(lt4b[64:65, :], 1.0)
                nc.scalar.copy(lt4b[0:64, :],
                               own_src[64:128, t * P:(t + 1) * P])

            # T' = x @ W2P + cP  (cP via the ones row)
            psT = ps_yt()
            if l < 4:
                nc.tensor.matmul(psT[:, 0:O], lhsT=lt[0:kk, :], rhs=w2_sb[l][:],
                                 start=True, stop=True)
            else:
                nc.tensor.matmul(psT[:, 0:O], lhsT=lt[0:64, :], rhs=w24a[:],
                                 start=True, stop=False)
                nc.tensor.matmul(psT[:, 0:O], lhsT=lt4b[0:65, :], rhs=w24b[:],
                                 start=False, stop=True)
            tsb = stg.tile([P, 256], F32, tag="tsb")
            nc.scalar.copy(tsb[:, 0:O], psT[:, 0:O])

            # pd row-block (f32r matmul: 1 cycle/row vs 4 for fp32; rank-only)
            def mdt(ap):
                return ap.bitcast(F32R) if USE_F32R else ap
            pd = pdp.tile([P, N], SCAN_DT, tag="pd")
            for cc in range(8):
                ppd = ps_pd()
                sl = slice(cc * 512, (cc + 1) * 512)
                if l < 4:
                    nc.tensor.matmul(ppd[:], lhsT=mdt(lt[0:kk, :]),
                                     rhs=mdt(xaug[0:kk, sl]),
                                     start=True, stop=True)
                else:
                    nc.tensor.matmul(ppd[:], lhsT=mdt(lt[0:64, :]),
                                     rhs=mdt(xaug[0:64, sl]),
                                     start=True, stop=False)
                    nc.tensor.matmul(ppd[:], lhsT=mdt(lt4b[0:65, :]),
                                     rhs=mdt(xaug4b[0:65, sl]),
                                     start=False, stop=True)
                nc.scalar.activation(pd[:, sl], ppd[:], AF.Identity,
                                     bias=negxx_t[:, t:t + 1], scale=1.0)

            # top-20 neighbour indices
            idxs = stg.tile([P, 24], U32, tag="idxs")
            if HIER:
                # hierarchical: top-8 per 512-col chunk (1 full-width pass
                # of max8 work), 64-wide merge scan, then 3 full-width
                # find_index8 passes on the untouched pd. ~4.1 full passes
                # instead of 8. A chunk holding >8 of the true top-20
                # (P~4e-4/row) costs a slightly-wrong neighbour set there.
                vals = stg.tile([P, 64], SCAN_DT, tag="vals")
                for c8 in range(8):
                    nc.vector.max(out=vals[:, c8 * 8:(c8 + 1) * 8],
                                  in_=pd[:, c8 * 512:(c8 + 1) * 512])
                m8 = stg.tile([P, 24], SCAN_DT, tag="m8")
                for r in range(3):
                    nc.vector.max(out=m8[:, r * 8:(r + 1) * 8], in_=vals[:])
                    if r < 2:
                        nc.vector.match_replace(
                            out=vals[:], in_to_replace=m8[:, r * 8:(r + 1) * 8],
                            in_values=vals[:], imm_value=SCAN_NEG)
                for r in range(3):
                    nc.vector.max_index(out=idxs[:, r * 8:(r + 1) * 8],
                                        in_max=m8[:, r * 8:(r + 1) * 8],
                                        in_values=pd[:])
            else:
                vals = stg.tile([P, 24], SCAN_DT, tag="vals")
                for r in range(3):
                    v8 = vals[:, r * 8:r * 8 + 8]
                    i8 = idxs[:, r * 8:r * 8 + 8]
                    nc.vector.max(out=v8, in_=pd[:])
                    nc.vector.max_index(out=i8, in_max=v8, in_values=pd[:])
                    if r < 2:
                        nc.vector.match_replace(out=pd[:], in_to_replace=v8,
                                                in_values=pd[:],
                                                imm_value=SCAN_NEG)

            # gather the 20 neighbour Y' rows, then max over k.  The swdge
            # walks the offset AP in ravel order, so a [128,20] offset AP
            # fetches all 20 rows per point in ONE ~1us instruction.
            G = gat.tile([P, 20 * O], Y_DT, tag="G")
            if GATHER1:
                nc.gpsimd.indirect_dma_start(
                    out=G[:, 0:20 * O], out_offset=None, in_=ydram[:],
                    in_offset=bass.IndirectOffsetOnAxis(ap=idxs[:, 0:20],
                                                        axis=0))
            else:
                for k in range(20):
                    nc.gpsimd.indirect_dma_start(
                        out=G[:, k * O:(k + 1) * O], out_offset=None,
                        in_=ydram[:],
                        in_offset=bass.IndirectOffsetOnAxis(ap=idxs[:, k:k + 1],
                                                            axis=0))
            Mx = stg.tile([P, 256], F32, tag="Mx")
            if REDGP:
                # packed fp16 max tree 20 -> 10 -> 5 -> (4+1) -> 1 chunks
                # of O on the DVE (2x mode; walrus rejects max on Pool)
                mt = gat.tile([P, 18 * 256], Y_DT, tag="mt")
                nc.vector.tensor_max(mt[:, 0:10 * O], G[:, 0:10 * O],
                                     G[:, 10 * O:20 * O])
                nc.vector.tensor_max(mt[:, 10 * O:15 * O], mt[:, 0:5 * O],
                                     mt[:, 5 * O:10 * O])
                nc.vector.tensor_max(mt[:, 15 * O:17 * O],
                                     mt[:, 10 * O:12 * O],
                                     mt[:, 12 * O:14 * O])
                nc.vector.tensor_max(mt[:, 17 * O:18 * O],
                                     mt[:, 15 * O:16 * O],
                                     mt[:, 16 * O:17 * O])
                nc.vector.tensor_max(Mx[:, 0:O], mt[:, 17 * O:18 * O],
                                     mt[:, 14 * O:15 * O])
            else:
                nc.vector.tensor_reduce(
                    out=Mx[:, 0:O],
                    in_=G[:].rearrange("p (s o) -> p o s", s=20, o=O),
                    axis=mybir.AxisListType.X, op=OP.max)

            # out = ReLU(Mx + T')
            pre = stg.tile([P, 256], F32, tag="pre")
            nc.gpsimd.tensor_add(pre[:, 0:O], Mx[:, 0:O], tsb[:, 0:O])
            opm = stg.tile([P, 256], F32, tag="opm")
            nc.scalar.activation(opm[:, 0:O], pre[:, 0:O], AF.Relu)

            # transpose to feature-major, retain own half, stage exchange
            if l < 4:
                ptr = ps_misc()
                nc.tensor.transpose(ptr[0:O, 0:P], opm[:, 0:O], ident[:])
                if l == 1:
                    dst = tileA[0:64, t * P:(t + 1) * P]
                elif l == 2:
                    dst = tileA[64:128, t * P:(t + 1) * P]
                else:
                    dst = tileB[0:128, t * P:(t + 1) * P]
                nc.scalar.copy(dst, ptr[0:O, 0:P])
                nc.sync.dma_start(eg_in[0:O, t * P:(t + 1) * P], dst)

                # -|x_new|^2/2 per point
                sq = stg.tile([P, 256], F32, tag="sq")
                xxc = stg.tile([P, 2], F32, tag="xxc")
                nc.scalar.activation(sq[:, 0:O], opm[:, 0:O], AF.Square,
                                     scale=float(math.sqrt(0.5)),
                                     accum_out=xxc[:, 0:1])
                nc.scalar.activation(xxc[:, 1:2], xxc[:, 0:1], AF.Identity,
                                     scale=-1.0)
                # next layer's per-own-tile bias, partition-major (no
                # transpose needed)
                nc.scalar.copy(negxx_next[:, t:t + 1], xxc[:, 1:2])
                pxr = ps_misc()
                nc.tensor.transpose(pxr[0:1, 0:P], xxc[:, 1:2], ident[:])
                nc.scalar.copy(xxrow_sb[:, t * P:(t + 1) * P], pxr[0:1, 0:P])
            else:
                ptr = ps_misc()
                nc.tensor.transpose(ptr[0:P, 0:P], opm[:, 0:128], ident[:])
                nc.scalar.copy(tileC[:, t * P:(t + 1) * P], ptr[0:P, 0:P])
                ptr2 = ps_misc()
                nc.tensor.transpose(ptr2[0:P, 0:P], opm[:, 128:256], ident[:])
                nc.scalar.copy(tileD[:, t * P:(t + 1) * P], ptr2[0:P, 0:P])

        # ---- stage C: exchange halves, assemble next layer's inputs ----
        if l < 4:
            nc.sync.dma_start(eg_in[O:O + 1, :], xxrow_sb[:])
            if SHARD_HALVES:
                nc.gpsimd.collective_compute(
                    "AllGather", OP.bypass,
                    replica_groups=[[0, 1], [2, 3], [4, 5], [6, 7]],
                    ins=[eg_in.opt()], outs=[eg_out.opt()])
                src, blocks = eg_out, [(0, 0), (O + 1, HALF)]
            else:
                src, blocks = eg_in, [(0, 0)]

            xaug_next = big.tile([P, N], F16, tag=xaug_tag[l + 1])
            if l < 3:
                for (srow, cbase) in blocks:
                    nc.sync.dma_start(xaug_next[0:O, cbase:cbase + HALF],
                                      src[srow:srow + O, :])
                    nc.sync.dma_start(xaug_next[O:O + 1, cbase:cbase + HALF],
                                      src[srow + O:srow + O + 1, :])
            else:
                # layer-4 rhs K-chunks: xaug4 rows 0:64 (via xaug_next) and
                # xaug4b = rows 64:128 plus the -xx/2 row
                xaug4b = big.tile([P, N], F16, tag="xaug_a")
                for (srow, cbase) in blocks:
                    nc.sync.dma_start(xaug_next[0:O, cbase:cbase + HALF],
                                      src[srow:srow + O, :])
                    nc.sync.dma_start(xaug4b[0:64, cbase:cbase + HALF],
                                      src[srow + 64:srow + 128, :])
                    nc.sync.dma_start(xaug4b[64:65, cbase:cbase + HALF],
                                      src[srow + O:srow + O + 1, :])

            negxx_t = negxx_next
            xaug = xaug_next

    # ---- MLP over retained feature-major tiles ----
    mw1 = const.tile([P, 4096], F16)
    nc.sync.dma_start(mw1[:], ins["mw1"][:])
    mc1 = const.tile([P, 8], F32)
    nc.sync.dma_start(mc1[:], ins["mc1"][:])
    mw2 = const.tile([P, 2048], F16)
    nc.sync.dma_start(mw2[:], ins["mw2"][:])
    mc2 = const.tile([P, 2], F32)
    nc.sync.dma_start(mc2[:], ins["mc2"][:])
    mw3 = const.tile([P, 4], F16)
    nc.sync.dma_start(mw3[:], ins["mw3"][:])
    mc3 = const.tile([2, 1], F32)
    nc.sync.dma_start(mc3[:], ins["mc3"][:])

    chunks = [tileA, tileB, tileC, tileD]
    for t in range(OWN_TILES):
        h1 = pdp.tile([P, 1024], F16, tag="pd", name="h1")
        for oc in range(8):
            ps = ps_pd()
            for kc in range(4):
                nc.tensor.matmul(
                    ps[:, 0:P],
                    lhsT=mw1[:, kc * 1024 + oc * P:kc * 1024 + (oc + 1) * P],
                    rhs=chunks[kc][:, t * P:(t + 1) * P],
                    start=(kc == 0), stop=(kc == 3))
            nc.scalar.activation(h1[:, oc * P:(oc + 1) * P], ps[:, 0:P],
                                 AF.Relu, bias=mc1[:, oc:oc + 1], scale=1.0)
        h2 = stg.tile([P, 256], F16, tag="h2")
        for oc in range(2):
            ps = ps_yt()
            for kc in range(8):
                nc.tensor.matmul(
                    ps[:, 0:P],
                    lhsT=mw2[:, kc * 256 + oc * P:kc * 256 + (oc + 1) * P],
                    rhs=h1[:, kc * P:(kc + 1) * P],
                    start=(kc == 0), stop=(kc == 7))
            nc.scalar.activation(h2[:, oc * P:(oc + 1) * P], ps[:, 0:P],
                                 AF.Relu, bias=mc2[:, oc:oc + 1], scale=1.0)
        ps3 = ps_misc()
        for kc in range(2):
            nc.tensor.matmul(ps3[0:2, 0:P], lhsT=mw3[:, kc * 2:(kc + 1) * 2],
                             rhs=h2[:, kc * P:(kc + 1) * P],
                             start=(kc == 0), stop=(kc == 1))
        osb = stg.tile([2, P], F32, tag="osb")
        nc.scalar.activation(osb[:], ps3[0:2, 0:P], AF.Identity,
                             bias=mc3[:], scale=1.0)
        nc.sync.dma_start(
            out_d[t * P:(t + 1) * P, :].rearrange("p c -> c p"), osb[:])


# --------------------------------------------------------------------------
# entry point
# --------------------------------------------------------------------------

_PROGRAM = None


def kernel(**inputs) -> np.ndarray:
    global _PROGRAM
    from concourse import bass_utils

    inputs = {k: np.asarray(v, np.float32) for k, v in inputs.items()}
    B = inputs["x"].shape[0]
    if _PROGRAM is None:
        _PROGRAM = build_program()
    nc = _PROGRAM
    in_maps = make_core_inputs(inputs)
    res = bass_utils.run_bass_kernel_spmd(nc, in_maps, core_ids=list(range(8)))
    outs = [r["out"] for r in res.results]
    full = np.empty((B, N, 2), np.float32)
    for c in range(8):
        b = (c // 2) % B
        h = c % 2
        if SHARD_HALVES:
            full[b, h * HALF:(h + 1) * HALF] = outs[c]
        elif c < B:
            full[c] = outs[c]
    return full


if __name__ == "__main__":
    import reference

    inputs = reference.setup_inputs()
    out = kernel(**{k: np.asarray(v) for k, v in inputs.items()})
    print(out.shape, out.dtype)



# revision 23
# speedup vs baseline: 1.3398x; 1.0076x over previous
"""DGCNN forward on 8 Trainium2 NeuronCores (Bass/Tile).

Contract: kernel(**inputs) takes the FULL unsharded inputs of
reference.setup_inputs() and returns the FULL (4, 4096, 2) output.

Sharding: cores (2b, 2b+1) handle batch b; each core computes half the
rows (2048) of every edge-conv layer (NxN knn + top-20 gather + max) and
of the point-wise MLP. Between layers the pair exchanges its half of the
transposed feature map (plus a -|x|^2/2 row) via a pairwise AllGather.

Math: eval-BN is affine with positive scale s, and ReLU/max commute with
positive affine maps, so each edge-conv collapses to
    out[i] = ReLU( max_{j in knn20(i)} Y[j] + T[i] )
with Y = X @ (s*Wa).T  (gathered by knn index: 20 rows per point via
indirect DMA, max-combined across two 10-row gathers) and
T = X @ (s*(Wb-Wa)).T + (s*(b-m)+beta)  from a small matmul.
knn ranking uses pd = inner - xx_i/2 - xx_j/2 (= reference pd / 2, same
ordering), computed on the PE; top-20 selection runs on the Vector
engine (max8 / max_index / match_replace rounds).
"""

import sys

sys.path.insert(0, "/opt/trn_rl_repo")

import math
import os
from contextlib import ExitStack

import numpy as np

import concourse.bass as bass
import concourse.tile as tile
from concourse import bacc, mybir
from concourse.masks import make_identity

EPS = 1e-5
K = 20
N = 4096
P = 128
NTILES = N // P            # 32 point tiles per batch
NEG = -3.0e38
F32 = mybir.dt.float32
U32 = mybir.dt.uint32
AF = mybir.ActivationFunctionType
OP = mybir.AluOpType

# (C_in, O_out) per edge-conv layer
DIMS = [(3, 64), (64, 64), (64, 128), (128, 256)]

# False: every core computes its full batch (no collectives) - debug mode.
SHARD_HALVES = os.environ.get("DGCNN_SHARD", "1") == "1"
TOPK_MODE = os.environ.get("DGCNN_TOPK", "flat")
USE_F32R = os.environ.get("DGCNN_F32R", "0") == "1"
SCAN16 = os.environ.get("DGCNN_SCAN16", "0") == "1"
HIER = os.environ.get("DGCNN_HIER", "1") == "1"
# multi-offset single-instruction gather: CoreSim supports it but real HW
# wedges (NRT_EXEC_UNIT_UNRECOVERABLE) — one dynamic offset per partition
# per instruction is a hardware limit. Keep off.
GATHER1 = os.environ.get("DGCNN_GATHER1", "0") == "1"
YF16 = os.environ.get("DGCNN_YF16", "1") == "1"
REDGP = os.environ.get("DGCNN_REDGP", "1") == "1"
F16 = mybir.dt.float16
F32R = mybir.dt.float32r
SCAN_DT = F16 if SCAN16 else mybir.dt.float32
SCAN_NEG = -60000.0 if SCAN16 else NEG
Y_DT = F16 if YF16 else mybir.dt.float32

HALF = N // 2 if SHARD_HALVES else N
OWN_TILES = HALF // P      # 16 (32 in debug mode)


# --------------------------------------------------------------------------
# host-side weight folding
# --------------------------------------------------------------------------

def _fold(inputs):
    d = {}
    for l, (C, O) in enumerate(DIMS, 1):
        w = inputs[f"cw{l}"]            # (O, 2C)
        b = inputs[f"cb{l}"]
        g, be = inputs[f"g{l}"], inputs[f"b{l}"]
        m, v = inputs[f"m{l}"], inputs[f"v{l}"]
        s = g / np.sqrt(v + EPS)
        Wa, Wb = w[:, :C], w[:, C:]
        WaP = (s[:, None] * Wa).T.astype(np.float32)        # (C, O)
        W2P = (s[:, None] * (Wb - Wa)).T.astype(np.float32)  # (C, O)
        cP = (s * (b - m) + be).astype(np.float32)           # (O,)
        d[f"wa{l}"] = np.ascontiguousarray(WaP.astype(np.float16))
        if l < 4:
            d[f"w2{l}"] = np.ascontiguousarray(
                np.concatenate([W2P, cP[None, :]], 0).astype(np.float16))
        else:
            d["w24a"] = np.ascontiguousarray(W2P[0:64].astype(np.float16))
            d["w24b"] = np.ascontiguousarray(
                np.concatenate([W2P[64:128], cP[None, :]],
                               0).astype(np.float16))        # (65, 256)

    def fold_lin(w, b, g, be, m, v):
        s = g / np.sqrt(v + EPS)
        return ((s[:, None] * w).T.astype(np.float32),
                (s * (b - m) + be).astype(np.float32))

    L1, c1 = fold_lin(inputs["l1w"], inputs["l1b"], inputs["f1g"],
                      inputs["f1b"], inputs["f1m"], inputs["f1v"])
    L2, c2 = fold_lin(inputs["l2w"], inputs["l2b"], inputs["f2g"],
                      inputs["f2b"], inputs["f2m"], inputs["f2v"])
    L3 = inputs["l3w"].T.astype(np.float32)                  # (256, 2)
    c3 = inputs["l3b"].astype(np.float32)

    d["mw1"] = np.ascontiguousarray(
        L1.reshape(4, 128, 1024).transpose(1, 0, 2).reshape(128, 4096)
        .astype(np.float16))
    d["mc1"] = np.ascontiguousarray(c1.reshape(8, 128).T)    # [128, 8]
    d["mw2"] = np.ascontiguousarray(
        L2.reshape(8, 128, 256).transpose(1, 0, 2).reshape(128, 2048)
        .astype(np.float16))
    d["mc2"] = np.ascontiguousarray(c2.reshape(2, 128).T)    # [128, 2]
    d["mw3"] = np.ascontiguousarray(
        L3.reshape(2, 128, 2).transpose(1, 0, 2).reshape(128, 4)
        .astype(np.float16))
    d["mc3"] = np.ascontiguousarray(c3[:, None])             # [2, 1]
    return d


def make_core_inputs(inputs):
    """Per-core input dicts (len 8). Core c -> batch c//2, half c%2."""
    shared = _fold(inputs)
    x = np.asarray(inputs["x"], np.float32)          # (B, 4096, 3)
    B = x.shape[0]
    per_core = []
    for c in range(8):
        b = (c // 2) % B if SHARD_HALVES else c % B
        h = c % 2 if SHARD_HALVES else 0
        xb = x[b]
        xx = (xb * xb).sum(-1)
        xaug = np.concatenate([xb.T, -0.5 * xx[None, :]], 0).astype(np.float32)
        d = dict(shared)
        d["xaug1"] = np.ascontiguousarray(xaug.astype(np.float16))   # (4, 4096)
        d["xown1"] = np.ascontiguousarray(
            xaug[:, h * HALF:(h + 1) * HALF].astype(np.float16))
        d["negxx1"] = np.ascontiguousarray(
            xaug[3, h * HALF:(h + 1) * HALF].reshape(HALF // 128, 128).T
            .astype(np.float32))
        per_core.append(d)
    return per_core


# --------------------------------------------------------------------------
# device program
# --------------------------------------------------------------------------

def build_program():
    nc = bacc.Bacc(None)
    ins = {}

    def einp(name, shape, dt=F16):
        ins[name] = nc.dram_tensor(name, shape, dt, kind="ExternalInput")
        return ins[name]

    einp("xaug1", [4, N])
    einp("xown1", [4, HALF])
    einp("negxx1", [128, HALF // 128], F32)
    for l, (C, O) in enumerate(DIMS, 1):
        einp(f"wa{l}", [C, O])
        if l < 4:
            einp(f"w2{l}", [C + 1, O])
    einp("w24a", [64, 256])
    einp("w24b", [65, 256])
    einp("mw1", [128, 4096])
    einp("mc1", [128, 8], F32)
    einp("mw2", [128, 2048])
    einp("mc2", [128, 2], F32)
    einp("mw3", [128, 4])
    einp("mc3", [2, 1], F32)

    out_d = nc.dram_tensor("out", [HALF, 2], F32, kind="ExternalOutput")

    with tile.TileContext(nc) as tc:
        with ExitStack() as ctx:
            _build_tile_graph(ctx, tc, ins, out_d)

    nc.compile()
    return nc


def _build_tile_graph(ctx, tc, ins, out_d):
    nc = tc.nc

    nb = 2 if SHARD_HALVES else 1
    const = ctx.enter_context(tc.tile_pool(name="const", bufs=1))
    big = ctx.enter_context(tc.tile_pool(name="big", bufs=1))
    pdp = ctx.enter_context(tc.tile_pool(name="pdp", bufs=3 if SHARD_HALVES else 1))
    stg = ctx.enter_context(tc.tile_pool(name="stg", bufs=nb))
    gat = ctx.enter_context(tc.tile_pool(name="gat", bufs=2))
    psum = ctx.enter_context(tc.tile_pool(name="psum", bufs=2, space="PSUM"))
    dram = ctx.enter_context(tc.tile_pool(name="dram", bufs=1, space="DRAM"))

    def ps_pd():
        return psum.tile([P, 512], F32, tag="pspd", name="pspd")

    def ps_yt():
        return psum.tile([P, 256], F32, tag="psY", name="psY")

    def ps_misc():
        return psum.tile([P, 512], F32, tag="psmisc", name="psmisc")

    ident = const.tile([P, P], F32)
    make_identity(nc, ident[:])

    wa_sb, w2_sb = {}, {}
    for l, (C, O) in enumerate(DIMS, 1):
        wa_sb[l] = const.tile([C, O], F16, tag=f"wa{l}", name=f"wa{l}sb")
        nc.sync.dma_start(wa_sb[l][:], ins[f"wa{l}"][:])
        if l < 4:
            w2_sb[l] = const.tile([C + 1, O], F16, tag=f"w2{l}", name=f"w2{l}sb")
            nc.sync.dma_start(w2_sb[l][:], ins[f"w2{l}"][:])
    w24a = const.tile([64, 256], F16)
    nc.sync.dma_start(w24a[:], ins["w24a"][:])
    w24b = const.tile([65, 256], F16)
    nc.sync.dma_start(w24b[:], ins["w24b"][:])

    # feature-major inputs: xaug rows 0..C-1 = x^T, row C = -|x|^2/2
    xaug_tag = {1: "xaug_a", 2: "xaug_b", 3: "xaug_a", 4: "xaug_b"}
    xaug = big.tile([P, N], F16, tag=xaug_tag[1])
    nc.sync.dma_start(xaug[0:4, :], ins["xaug1"][:])
    xown1 = big.tile([P, HALF], F16, tag="ownD")
    nc.sync.dma_start(xown1[0:4, :], ins["xown1"][:])

    # retained own-half feature-major activations (MLP k-chunks)
    tileA = big.tile([P, HALF], F16, tag="mlpA")  # x1T (rows 0:64) | x2T (64:128)
    tileB = big.tile([P, HALF], F16, tag="mlpB")  # x3T
    tileC = big.tile([P, HALF], F16, tag="mlpC")  # x4T rows 0:128
    tileD = big.tile([P, HALF], F16, tag="ownD")  # x4T rows 128:256 (reuses xown1)
    # (tile, row_offset) holding each layer's own-half input, feature-major
    xown_map = {1: (xown1, 0), 2: (tileA, 0), 3: (tileA, 64), 4: (tileB, 0)}

    negxx_t = None        # [P, OWN_TILES] per-tile -xx/2 bias (current layer)
    xaug4b = None

    for l, (C, O) in enumerate(DIMS, 1):
        # ---- stage A: Y' for the whole batch -> DRAM ----
        ydram = dram.tile([N, O], Y_DT, tag=f"yd{l}")
        for j in range(NTILES):
            ps = ps_yt()
            nc.tensor.matmul(ps[:, 0:O], lhsT=xaug[0:C, j * P:(j + 1) * P],
                             rhs=wa_sb[l][:], start=True, stop=True)
            ysb = stg.tile([P, 256], Y_DT, tag="ysb")
            nc.scalar.copy(ysb[:, 0:O], ps[:, 0:O])
            nc.sync.dma_start(ydram[j * P:(j + 1) * P, :], ysb[:, 0:O])

        if l == 1:
            negxx_t = stg.tile([P, NTILES], F32, tag="negxx")
            nc.sync.dma_start(negxx_t[:, 0:OWN_TILES], ins["negxx1"][:])

        if l < 4:
            eg_in = dram.tile([O + 1, HALF], F16, tag=f"egin{l}")
            if SHARD_HALVES:
                eg_out = dram.tile([2 * (O + 1), HALF], F16, tag=f"egout{l}")
            xxrow_sb = big.tile([1, HALF], F16, tag="xxrow")
            negxx_next = stg.tile([P, NTILES], F32, tag="negxx")

        own_src, own_ro = xown_map[l]

        # ---- stage B: per own row-tile ----
        for t in range(OWN_TILES):
            # lhsT staging: rows 0..C-1 = x^T own slice, row C = ones
            lt = stg.tile([P, P], F16, tag="lhsT")
            if C < 32:
                nc.gpsimd.memset(lt[0:32, :], 1.0)
            elif l < 4:
                nc.gpsimd.memset(lt[C:C + 1, :], 1.0)
            nc.scalar.copy(lt[0:C if l < 4 else 64, :],
                           own_src[own_ro:own_ro + (C if l < 4 else 64),
                                   t * P:(t + 1) * P])
            kk = C + 1 if l < 4 else 64
            if l == 4:
                # K-split: second chunk carries rows 64:128 plus a ones row
                # (for cP / the -xx_j/2 row on the rhs side)
                lt4b = stg.tile([P, P], F16, tag="lhsT4b")
                nc.gpsimd.memset(lt4b[64:65, :], 1.0)
                nc.scalar.copy(lt4b[0:64, :],
                               own_src[64:128, t * P:(t + 1) * P])

            # T' = x @ W2P + cP  (cP via the ones row)
            psT = ps_yt()
            if l < 4:
                nc.tensor.matmul(psT[:, 0:O], lhsT=lt[0:kk, :], rhs=w2_sb[l][:],
                                 start=True, stop=True)
            else:
                nc.tensor.matmul(psT[:, 0:O], lhsT=lt[0:64, :], rhs=w24a[:],
                                 start=True, stop=False)
                nc.tensor.matmul(psT[:, 0:O], lhsT=lt4b[0:65, :], rhs=w24b[:],
                                 start=False, stop=True)
            tsb = stg.tile([P, 256], F32, tag="tsb")
            nc.scalar.copy(tsb[:, 0:O], psT[:, 0:O])

            # pd row-block (f32r matmul: 1 cycle/row vs 4 for fp32; rank-only)
            def mdt(ap):
                return ap.bitcast(F32R) if USE_F32R else ap
            pd = pdp.tile([P, N], SCAN_DT, tag="pd")
            for cc in range(8):
                ppd = ps_pd()
                sl = slice(cc * 512, (cc + 1) * 512)
                if l < 4:
                    nc.tensor.matmul(ppd[:], lhsT=mdt(lt[0:kk, :]),
                                     rhs=mdt(xaug[0:kk, sl]),
                                     start=True, stop=True)
                else:
                    nc.tensor.matmul(ppd[:], lhsT=mdt(lt[0:64, :]),
                                     rhs=mdt(xaug[0:64, sl]),
                                     start=True, stop=False)
                    nc.tensor.matmul(ppd[:], lhsT=mdt(lt4b[0:65, :]),
                                     rhs=mdt(xaug4b[0:65, sl]),
                                     start=False, stop=True)
                nc.scalar.activation(pd[:, sl], ppd[:], AF.Identity,
                                     bias=negxx_t[:, t:t + 1], scale=1.0)

            # top-20 neighbour indices
            idxs = stg.tile([P, 24], U32, tag="idxs")
            if HIER:
                # hierarchical: top-8 per 512-col chunk (1 full-width pass
                # of max8 work), 64-wide merge scan, then 3 full-width
                # find_index8 passes on the untouched pd. ~4.1 full passes
                # instead of 8. A chunk holding >8 of the true top-20
                # (P~4e-4/row) costs a slightly-wrong neighbour set there.
                vals = stg.tile([P, 64], SCAN_DT, tag="vals")
                for c8 in range(8):
                    nc.vector.max(out=vals[:, c8 * 8:(c8 + 1) * 8],
                                  in_=pd[:, c8 * 512:(c8 + 1) * 512])
                m8 = stg.tile([P, 24], SCAN_DT, tag="m8")
                for r in range(3):
                    nc.vector.max(out=m8[:, r * 8:(r + 1) * 8], in_=vals[:])
                    if r < 2:
                        nc.vector.match_replace(
                            out=vals[:], in_to_replace=m8[:, r * 8:(r + 1) * 8],
                            in_values=vals[:], imm_value=SCAN_NEG)
                for r in range(3):
                    nc.vector.max_index(out=idxs[:, r * 8:(r + 1) * 8],
                                        in_max=m8[:, r * 8:(r + 1) * 8],
                                        in_values=pd[:])
            else:
                vals = stg.tile([P, 24], SCAN_DT, tag="vals")
                for r in range(3):
                    v8 = vals[:, r * 8:r * 8 + 8]
                    i8 = idxs[:, r * 8:r * 8 + 8]
                    nc.vector.max(out=v8, in_=pd[:])
                    nc.vector.max_index(out=i8, in_max=v8, in_values=pd[:])
                    if r < 2:
                        nc.vector.match_replace(out=pd[:], in_to_replace=v8,
                                                in_values=pd[:],
                                                imm_value=SCAN_NEG)

            # gather the 20 neighbour Y' rows, then max over k.  The swdge
            # walks the offset AP in ravel order, so a [128,20] offset AP
            # fetches all 20 rows per point in ONE ~1us instruction.
            G = gat.tile([P, 20 * O], Y_DT, tag="G")
            if GATHER1:
                nc.gpsimd.indirect_dma_start(
                    out=G[:, 0:20 * O], out_offset=None, in_=ydram[:],
                    in_offset=bass.IndirectOffsetOnAxis(ap=idxs[:, 0:20],
                                                        axis=0))
            else:
                for k in range(20):
                    nc.gpsimd.indirect_dma_start(
                        out=G[:, k * O:(k + 1) * O], out_offset=None,
                        in_=ydram[:],
                        in_offset=bass.IndirectOffsetOnAxis(ap=idxs[:, k:k + 1],
                                                            axis=0))
            Mx = stg.tile([P, 256], F32, tag="Mx")
            if REDGP:
                # packed fp16 max tree 20 -> 10 -> 5 -> (4+1) -> 1 chunks
                # of O on the DVE (2x mode; walrus rejects max on Pool)
                mt = gat.tile([P, 18 * 256], Y_DT, tag="mt")
                nc.vector.tensor_max(mt[:, 0:10 * O], G[:, 0:10 * O],
                                     G[:, 10 * O:20 * O])
                nc.vector.tensor_max(mt[:, 10 * O:15 * O], mt[:, 0:5 * O],
                                     mt[:, 5 * O:10 * O])
                nc.vector.tensor_max(mt[:, 15 * O:17 * O],
                                     mt[:, 10 * O:12 * O],
                                     mt[:, 12 * O:14 * O])
                nc.vector.tensor_max(mt[:, 17 * O:18 * O],
                                     mt[:, 15 * O:16 * O],
                                     mt[:, 16 * O:17 * O])
                nc.vector.tensor_max(Mx[:, 0:O], mt[:, 17 * O:18 * O],
                                     mt[:, 14 * O:15 * O])
            else:
                nc.vector.tensor_reduce(
                    out=Mx[:, 0:O],
                    in_=G[:].rearrange("p (s o) -> p o s", s=20, o=O),
                    axis=mybir.AxisListType.X, op=OP.max)

            # out = ReLU(Mx + T')
            pre = stg.tile([P, 256], F32, tag="pre")
            nc.gpsimd.tensor_add(pre[:, 0:O], Mx[:, 0:O], tsb[:, 0:O])
            opm = stg.tile([P, 256], F32, tag="opm")
            nc.scalar.activation(opm[:, 0:O], pre[:, 0:O], AF.Relu)

            # transpose to feature-major, retain own half, stage exchange
            if l < 4:
                ptr = ps_misc()
                nc.tensor.transpose(ptr[0:O, 0:P], opm[:, 0:O], ident[:])
                if l == 1:
                    dst = tileA[0:64, t * P:(t + 1) * P]
                elif l == 2:
                    dst = tileA[64:128, t * P:(t + 1) * P]
                else:
                    dst = tileB[0:128, t * P:(t + 1) * P]
                nc.scalar.copy(dst, ptr[0:O, 0:P])
                nc.sync.dma_start(eg_in[0:O, t * P:(t + 1) * P], dst)

                # -|x_new|^2/2 per point
                sq = stg.tile([P, 256], F32, tag="sq")
                xxc = stg.tile([P, 2], F32, tag="xxc")
                nc.scalar.activation(sq[:, 0:O], opm[:, 0:O], AF.Square,
                                     scale=float(math.sqrt(0.5)),
                                     accum_out=xxc[:, 0:1])
                nc.scalar.activation(xxc[:, 1:2], xxc[:, 0:1], AF.Identity,
                                     scale=-1.0)
                # next layer's per-own-tile bias, partition-major (no
                # transpose needed)
                nc.scalar.copy(negxx_next[:, t:t + 1], xxc[:, 1:2])
                pxr = ps_misc()
                nc.tensor.transpose(pxr[0:1, 0:P], xxc[:, 1:2], ident[:])
                nc.scalar.copy(xxrow_sb[:, t * P:(t + 1) * P], pxr[0:1, 0:P])
            else:
                ptr = ps_misc()
                nc.tensor.transpose(ptr[0:P, 0:P], opm[:, 0:128], ident[:])
                nc.scalar.copy(tileC[:, t * P:(t + 1) * P], ptr[0:P, 0:P])
                ptr2 = ps_misc()
                nc.tensor.transpose(ptr2[0:P, 0:P], opm[:, 128:256], ident[:])
                nc.scalar.copy(tileD[:, t * P:(t + 1) * P], ptr2[0:P, 0:P])

        # ---- stage C: exchange halves, assemble next layer's inputs ----
        if l < 4:
            nc.sync.dma_start(eg_in[O:O + 1, :], xxrow_sb[:])
            if SHARD_HALVES:
                nc.gpsimd.collective_compute(
                    "AllGather", OP.bypass,
                    replica_groups=[[0, 1], [2, 3], [4, 5], [6, 7]],
                    ins=[eg_in.opt()], outs=[eg_out.opt()])
                src, blocks = eg_out, [(0, 0), (O + 1, HALF)]
            else:
                src, blocks = eg_in, [(0, 0)]

            xaug_next = big.tile([P, N], F16, tag=xaug_tag[l + 1])
            if l < 3:
                for (srow, cbase) in blocks:
                    nc.sync.dma_start(xaug_next[0:O, cbase:cbase + HALF],
                                      src[srow:srow + O, :])
                    nc.sync.dma_start(xaug_next[O:O + 1, cbase:cbase + HALF],
                                      src[srow + O:srow + O + 1, :])
            else:
                # layer-4 rhs K-chunks: xaug4 rows 0:64 (via xaug_next) and
                # xaug4b = rows 64:128 plus the -xx/2 row
                xaug4b = big.tile([P, N], F16, tag="xaug_a")
                for (srow, cbase) in blocks:
                    nc.sync.dma_start(xaug_next[0:O, cbase:cbase + HALF],
                                      src[srow:srow + O, :])
                    nc.sync.dma_start(xaug4b[0:64, cbase:cbase + HALF],
                                      src[srow + 64:srow + 128, :])
                    nc.sync.dma_start(xaug4b[64:65, cbase:cbase + HALF],
                                      src[srow + O:srow + O + 1, :])

            negxx_t = negxx_next
            xaug = xaug_next

    # ---- MLP over retained feature-major tiles ----
    mw1 = const.tile([P, 4096], F16)
    nc.sync.dma_start(mw1[:], ins["mw1"][:])
    mc1 = const.tile([P, 8], F32)
    nc.sync.dma_start(mc1[:], ins["mc1"][:])
    mw2 = const.tile([P, 2048], F16)
    nc.sync.dma_start(mw2[:], ins["mw2"][:])
    mc2 = const.tile([P, 2], F32)
    nc.sync.dma_start(mc2[:], ins["mc2"][:])
    mw3 = const.tile([P, 4], F16)
    nc.sync.dma_start(mw3[:], ins["mw3"][:])
    mc3 = const.tile([2, 1], F32)
    nc.sync.dma_start(mc3[:], ins["mc3"][:])

    chunks = [tileA, tileB, tileC, tileD]
    for t in range(OWN_TILES):
        h1 = pdp.tile([P, 1024], F16, tag="pd", name="h1")
        for oc in range(8):
            ps = ps_pd()
            for kc in range(4):
                nc.tensor.matmul(
                    ps[:, 0:P],
                    lhsT=mw1[:, kc * 1024 + oc * P:kc * 1024 + (oc + 1) * P],
                    rhs=chunks[kc][:, t * P:(t + 1) * P],
                    start=(kc == 0), stop=(kc == 3))
            nc.scalar.activation(h1[:, oc * P:(oc + 1) * P], ps[:, 0:P],
                                 AF.Relu, bias=mc1[:, oc:oc + 1], scale=1.0)
        h2 = stg.tile([P, 256], F16, tag="h2")
        for oc in range(2):
            ps = ps_yt()
            for kc in range(8):
                nc.tensor.matmul(
                    ps[:, 0:P],
                    lhsT=mw2[:, kc * 256 + oc * P:kc * 256 + (oc + 1) * P],
                    rhs=h1[:, kc * P:(kc + 1) * P],
                    start=(kc == 0), stop=(kc == 7))
            nc.scalar.activation(h2[:, oc * P:(oc + 1) * P], ps[:, 0:P],
                                 AF.Relu, bias=mc2[:, oc:oc + 1], scale=1.0)
        ps3 = ps_misc()
        for kc in range(2):
            nc.tensor.matmul(ps3[0:2, 0:P], lhsT=mw3[:, kc * 2:(kc + 1) * 2],
                             rhs=h2[:, kc * P:(kc + 1) * P],
                             start=(kc == 0), stop=(kc == 1))
        osb = stg.tile([2, P], F32, tag="osb")
        nc.scalar.activation(osb[:], ps3[0:2, 0:P], AF.Identity,
                             bias=mc3[:], scale=1.0)
        nc.sync.dma_start(
            out_d[t * P:(t + 1) * P, :].rearrange("p c -> c p"), osb[:])


# --------------------------------------------------------------------------
# entry point
# --------------------------------------------------------------------------

_PROGRAM = None


def kernel(**inputs) -> np.ndarray:
    global _PROGRAM
    from concourse import bass_utils

    inputs = {k: np.asarray(v, np.float32) for k, v in inputs.items()}
    B = inputs["x"].shape[0]
    if _PROGRAM is None:
        _PROGRAM = build_program()
    nc = _PROGRAM
    in_maps = make_core_inputs(inputs)
    res = bass_utils.run_bass_kernel_spmd(nc, in_maps, core_ids=list(range(8)))
    outs = [r["out"] for r in res.results]
    full = np.empty((B, N, 2), np.float32)
    for c in range(8):
        b = (c // 2) % B
        h = c % 2
        if SHARD_HALVES:
            full[b, h * HALF:(h + 1) * HALF] = outs[c]
        elif c < B:
            full[c] = outs[c]
    return full


if __name__ == "__main__":
    import reference

    inputs = reference.setup_inputs()
    out = kernel(**{k: np.asarray(v) for k, v in inputs.items()})
    print(out.shape, out.dtype)

